# revision 47
# baseline (speedup 1.0000x reference)
"""Causal group-query attention on 8 Trainium2 NeuronCores (bf16 edition).

Sharding: core c -> (batch b = c // 4, kv-group g = c % 4).
Each core owns batch element b, q-heads [4g, 4g+4) and kv-group g (n_rep = 4,
so those 4 q-heads attend to exactly kv-group g's k/v).  Every core computes
its partial o_proj output (contracting head-concat columns [512g, 512g+512)),
and the host sums the 4 partials per batch element (the "all-reduce after
o_proj" done host-side since we return full outputs anyway).

v2 vs the fp32r baseline (394us):
  * all matmuls in bf16: moving operand streams 2 elem/cycle (vs 1 for
    fp32r) and FWL kicks in for the 128-col weight loads -> ~2x PE.
  * PSUM laid out as 2 wide [128,2,512] tiles + 4 narrow [128,512] tiles
    (8 banks total).  Phase A: qt01/qt23 in the wide tiles, kt/vt narrow.
    Phase B: the wide tiles double-buffer the head-pair score banks, so
    one ACT instruction computes exp for BOTH heads of a pair ([128,2,N]
    3-D AP), halving ACT's 352-cycle per-instruction overhead.
  * both heads' softmax denominators accumulate into ONE narrow bank at
    partitions 0/32 (tile_position=(0,32j)); only the first matmul of the
    pair uses start=True, so the whole-bank has_written clear happens
    exactly once.
  * reciprocal via reciprocal_approx_fast ([1,512], ~5x faster than
    nc.vector.reciprocal) straight from PSUM; the raw->recip->PE
    broadcast->DVE multiply chain never touches ACT.
  * q/k/v bias-adds evacuate PSUM on DVE (tensor_scalar add) instead of
    ACT activations; causal masking runs on the otherwise-idle GpSimd.
  * o_proj of block tb-1 is emitted interleaved with block tb's RoPE so
    the PE always has ready work while ACT/DVE chew the serial chains.
"""

import math

import numpy as np

B, T, D = 2, 2048, 2048
N_HEAD, N_GROUP = 16, 4
HS = D // N_HEAD  # 128
N_REP = N_HEAD // N_GROUP  # 4
NH_C = N_HEAD // N_GROUP  # heads per core = 4
INV_SQRT_HS = 1.0 / math.sqrt(HS)

_NC_CACHE: dict = {}


def build_nc(t=T, dt16="bf16", den_pack=False, recip_psum=True, gp_mask=True,
             pair_oproj=True, den_tile=False):
    """Build and compile the per-core Bass program. Returns the compiled nc."""
    import concourse.bass as bass  # noqa: F401
    import concourse.mybir as mybir
    import concourse.tile as tile
    from concourse import bacc

    f32 = mybir.dt.float32
    f32r = mybir.dt.float32r
    DT = {"bf16": mybir.dt.bfloat16, "fp16": mybir.dt.float16}[dt16]
    exp_f = mybir.ActivationFunctionType.Exp
    ident_f = mybir.ActivationFunctionType.Identity

    # pair_oproj needs the nar-ring slot den_pack frees up; without it the
    # ring WAR chain (op unit -> ot read -> bc matmul) can deadlock.
    pair_oproj = pair_oproj and den_pack

    nd = D // 128  # d-tiles (contraction) = 16
    tb_n = t // 512  # 512-wide t blocks
    nk = t // 128  # 128-wide k tiles

    nc = bacc.Bacc("TRN2", target_bir_lowering=False, debug=False)

    xd = nc.dram_tensor("x_t", [D, t], DT, kind="ExternalInput")
    wqd = nc.dram_tensor("wq_t", [D, NH_C * HS], DT, kind="ExternalInput")
    wkd = nc.dram_tensor("wk_t", [D, HS], DT, kind="ExternalInput")
    wvd = nc.dram_tensor("wv_t", [D, HS], DT, kind="ExternalInput")
    wod = nc.dram_tensor("wo_t", [NH_C * HS, D], DT, kind="ExternalInput")
    cosd = nc.dram_tensor("cos_t", [HS, t], DT, kind="ExternalInput")
    sind = nc.dram_tensor("sin_t", [HS, t], DT, kind="ExternalInput")
    bqd = nc.dram_tensor("b_q", [HS, NH_C], f32, kind="ExternalInput")
    bkd = nc.dram_tensor("b_k", [HS, 1], f32, kind="ExternalInput")
    bvd = nc.dram_tensor("b_v", [HS, 1], f32, kind="ExternalInput")
    rtd = nc.dram_tensor("r_t", [HS, HS], DT, kind="ExternalInput")
    maskd = nc.dram_tensor("mask_ut", [128, 128], DT, kind="ExternalInput")
    identd = nc.dram_tensor("ident", [128, 128], f32, kind="ExternalInput")
    outd = nc.dram_tensor("out", [t, D], f32, kind="ExternalOutput")

    with tile.TileContext(nc) as tc:
        with (
            tc.tile_pool(name="consts", bufs=1) as consts,
            tc.tile_pool(name="wpool", bufs=1) as wpool,
            tc.tile_pool(name="resid", bufs=1) as resid,
            tc.tile_pool(name="xin", bufs=10) as xin,
            tc.tile_pool(name="work", bufs=4) as work,
            tc.tile_pool(name="vwork", bufs=2) as vwork,
            tc.tile_pool(name="drow", bufs=4) as drow,
            tc.tile_pool(name="qfp", bufs=2) as qfp,
            tc.tile_pool(name="ptp", bufs=4) as ptp,
            tc.tile_pool(name="otp", bufs=8) as otp,
            tc.tile_pool(name="bcp", bufs=2) as bcp,
            tc.tile_pool(name="outp", bufs=4) as outp,
            tc.tile_pool(name="psw", bufs=2, space="PSUM") as psw,
            tc.tile_pool(name="psn", bufs=4, space="PSUM") as psn,
        ):
            def wide(name):
                return psw.tile([128, 2, 512], f32, tag="wide", name=name)

            def nar(name):
                return psn.tile([128, 512], f32, tag="nar", name=name)

            # ---- constants / weights (loaded once) ----
            cos_sb = consts.tile([128, t], DT, name="cos_sb")
            sin_sb = consts.tile([128, t], DT, name="sin_sb")
            rt_sb = consts.tile([128, 128], DT, name="rt_sb")
            mask_sb = consts.tile([128, 128], DT, name="mask_sb")
            id_sb = consts.tile([128, 128], f32, name="id_sb")
            ones16 = consts.tile([128, 1], DT, name="ones16")
            onesf_raw = consts.tile([1, 128], f32, name="onesf_raw")
            onesf = consts.tile([1, 128], f32, name="onesf")
            bq_sb = consts.tile([128, NH_C], f32, name="bq_sb")
            bk_sb = consts.tile([128, 1], f32, name="bk_sb")
            bv_sb = consts.tile([128, 1], f32, name="bv_sb")
            wq_sb = wpool.tile([128, nd, NH_C * HS], DT, name="wq_sb")
            wk_sb = wpool.tile([128, nd, HS], DT, name="wk_sb")
            wv_sb = wpool.tile([128, nd, HS], DT, name="wv_sb")
            wo_sb = wpool.tile([128, NH_C, D], DT, name="wo_sb")
            wq_re = wqd[:, :].rearrange("(n p) m -> p n m", p=128)
            wk_re = wkd[:, :].rearrange("(n p) m -> p n m", p=128)
            wv_re = wvd[:, :].rearrange("(n p) m -> p n m", p=128)

            # resident K^T [hs, t] and V [t(128-tiles), hs]
            kt_sb = resid.tile([128, t], DT, name="kt_sb")
            v_sb = resid.tile([128, nk, HS], DT, name="v_sb")

            x_re = xd[:, :].rearrange("(n p) t -> p n t", p=128)

            oproj_units = []  # deferred (tb, s, ot_sb) work units
            oproj_q = [0]  # alternate output-DMA queue (gpsimd / sync)

            def emit_oproj_unit(evac_eng, split_dma=False):
                # one unit = a full 128-row output block: 4 db sub-blocks of
                # 4 head-accumulated matmuls each, evacuated into one wide
                # [128, 2048] tile and shipped with a single DMA.  With
                # split_dma (tail units) each db half ships separately so the
                # transfer overlaps the remaining evacuations.
                if not oproj_units:
                    return
                tb, s, ot_sb = oproj_units.pop(0)
                ts0 = tb * 512
                ob = outp.tile([128, D], f32, name="ob")
                # db-pairs with h outer: each head's stationary osb slice is
                # loaded once per pair of db sub-blocks instead of per block.
                for dbp in range(2):
                    op_ps = {db: nar("op_ps") for db in (2 * dbp, 2 * dbp + 1)}
                    for h in range(NH_C):
                        for db in (2 * dbp, 2 * dbp + 1):
                            nc.tensor.matmul(
                                op_ps[db],
                                lhsT=ot_sb[h][:, 128 * s: 128 * (s + 1)],
                                rhs=wo_sb[:, h, 512 * db: 512 * (db + 1)],
                                start=h == 0, stop=h == NH_C - 1,
                            )
                    for db in (2 * dbp, 2 * dbp + 1):
                        ob_sl = ob[:, 512 * db: 512 * (db + 1)]
                        if (db % 2 == 0) == (evac_eng == "v"):
                            nc.vector.tensor_copy(out=ob_sl, in_=op_ps[db])
                        else:
                            nc.scalar.copy(out=ob_sl, in_=op_ps[db])
                    if split_dma:
                        eng = nc.gpsimd if oproj_q[0] % 2 == 0 else nc.sync
                        oproj_q[0] += 1
                        eng.dma_start(
                            out=outd[
                                ts0 + 128 * s: ts0 + 128 * (s + 1),
                                1024 * dbp: 1024 * (dbp + 1),
                            ],
                            in_=ob[:, 1024 * dbp: 1024 * (dbp + 1)],
                        )
                if not split_dma:
                    eng = nc.gpsimd if oproj_q[0] % 2 == 0 else nc.sync
                    oproj_q[0] += 1
                    eng.dma_start(
                        out=outd[ts0 + 128 * s: ts0 + 128 * (s + 1), :],
                        in_=ob,
                    )

            def emit_proj(tb):
                # ============ stage 1: q/k/v projection matmuls for block tb
                ts0 = tb * 512
                qt01 = wide("qt01")
                qt23 = wide("qt23")
                kt_ps = nar("kt_ps")
                vt_ps = nar("vt_ps")
                xts_tb = []
                for chunk in range(nd // 2):
                    c2 = 2 * chunk
                    xt = xin.tile([128, 2, 512], DT, name="xt")
                    xts_tb.append(xt)
                    nc.sync.dma_start(
                        out=xt, in_=x_re[:, c2: c2 + 2, ts0: ts0 + 512]
                    )
                    if tb == 0:
                        nc.sync.dma_start(
                            out=wq_sb[:, c2: c2 + 2, :],
                            in_=wq_re[:, c2: c2 + 2, :],
                        )
                        nc.sync.dma_start(
                            out=wk_sb[:, c2: c2 + 2, :],
                            in_=wk_re[:, c2: c2 + 2, :],
                        )
                        nc.sync.dma_start(
                            out=wv_sb[:, c2: c2 + 2, :],
                            in_=wv_re[:, c2: c2 + 2, :],
                        )
                        if chunk == 0:
                            nc.sync.dma_start(out=bq_sb, in_=bqd[:, :])
                            nc.sync.dma_start(out=bk_sb, in_=bkd[:, :])
                            nc.sync.dma_start(out=bv_sb, in_=bvd[:, :])
                            nc.vector.memset(ones16, 1.0)
                            nc.vector.memset(onesf_raw, 1.0)
                            # fp32r matmul operands must come from a
                            # producer with f32r-typed output (walrus rule)
                            nc.vector.tensor_copy(
                                out=onesf[:, :].bitcast(f32r), in_=onesf_raw
                            )
                    for j in range(2):
                        dt = c2 + j
                        first, last = dt == 0, dt == nd - 1
                        for h in range(2):
                            nc.tensor.matmul(
                                qt01[:, h, :],
                                lhsT=wq_sb[:, dt, h * HS: (h + 1) * HS],
                                rhs=xt[:, j, :],
                                start=first, stop=last,
                            )
                        for h in range(2):
                            nc.tensor.matmul(
                                qt23[:, h, :],
                                lhsT=wq_sb[:, dt, (2 + h) * HS: (3 + h) * HS],
                                rhs=xt[:, j, :],
                                start=first, stop=last,
                            )
                        nc.tensor.matmul(
                            kt_ps, lhsT=wk_sb[:, dt, :], rhs=xt[:, j, :],
                            start=first, stop=last,
                        )
                        nc.tensor.matmul(
                            vt_ps, lhsT=wv_sb[:, dt, :], rhs=xt[:, j, :],
                            start=first, stop=last,
                        )

                if tb == 0:
                    # one-time loads go on the gpsimd queue so the sync queue
                    # stays dedicated to the xt stream.  Gate the queue on
                    # chunk 3's arrival so these bulk loads don't steal HBM
                    # bandwidth from the critical first x/w chunks.
                    gate = drow.tile([1, 1], DT, name="gate")
                    nc.gpsimd.tensor_copy(
                        out=gate, in_=xts_tb[3][0:1, 0, 0:1]
                    )
                    nc.gpsimd.dma_start(out=rt_sb, in_=rtd[:, :])
                    nc.gpsimd.dma_start(out=id_sb, in_=identd[:, :])
                    nc.gpsimd.dma_start(out=mask_sb, in_=maskd[:, :])
                    nc.gpsimd.dma_start(out=cos_sb, in_=cosd[:, :])
                    nc.gpsimd.dma_start(out=sin_sb, in_=sind[:, :])
                    wo_re = wod[:, :].rearrange("(h p) m -> p h m", p=128)
                    for h in range(NH_C):
                        nc.gpsimd.dma_start(
                            out=wo_sb[:, h: h + 1, :],
                            in_=wo_re[:, h: h + 1, :],
                        )
                return dict(qt01=qt01, qt23=qt23, kt_ps=kt_ps, vt_ps=vt_ps)

            def emit_evac(tb, stt):
                # ============ stage 2: bias-add PSUM evacuations.  k/v on
                # DVE (the k-rope chain is the critical path), q on ACT so
                # the two chains overlap.  Frees all of stage 1's banks.
                kraw = work.tile([128, 512], DT, name="kraw", bufs=2)
                nc.vector.tensor_scalar_add(
                    out=kraw, in0=stt["kt_ps"], scalar1=bk_sb[:, 0:1]
                )
                vraw = vwork.tile([128, 512], f32, name="vraw")
                nc.vector.tensor_scalar_add(
                    out=vraw, in0=stt["vt_ps"], scalar1=bv_sb[:, 0:1]
                )
                qraws = []
                for h in range(NH_C):
                    qraw = work.tile([128, 512], DT, name="qraw")
                    src = (stt["qt01"][:, h, :] if h < 2
                           else stt["qt23"][:, h - 2, :])
                    nc.scalar.activation(
                        out=qraw, in_=src, func=ident_f,
                        bias=bq_sb[:, h: h + 1], scale=1.0,
                    )
                    qraws.append(qraw)
                stt.update(kraw=kraw, vraw=vraw, qraws=qraws)

            def emit_rope(tb, stt):
                # ============ stage 3: RoPE (k then q) + V transpose.
                # o_proj units of the previous block keep the PE fed while
                # ACT/DVE chew the serial chains.
                ts0 = tb * 512
                kraw, vraw, qraws = stt["kraw"], stt["vraw"], stt["qraws"]
                qf = qfp.tile([128, NH_C, 512], DT, name="qf")
                rot_k = nar("rot_ps")
                nc.tensor.matmul(
                    rot_k, lhsT=rt_sb[:, :], rhs=kraw, start=True, stop=True
                )
                emit_oproj_unit("s")
                nc.vector.tensor_mul(
                    kt_sb[:, ts0: ts0 + 512], kraw, cos_sb[:, ts0: ts0 + 512]
                )
                rtmp = work.tile([128, 512], DT, name="rtmp", bufs=2)
                nc.vector.tensor_mul(rtmp, rot_k, sin_sb[:, ts0: ts0 + 512])
                nc.vector.tensor_add(
                    kt_sb[:, ts0: ts0 + 512], kt_sb[:, ts0: ts0 + 512], rtmp
                )

                # v: transpose [hs, t] -> [t, hs] tiles
                for s in range(4):
                    vt_tp = nar("vt_tp")
                    nc.tensor.transpose(
                        vt_tp[:, 0:128], vraw[:, 128 * s: 128 * (s + 1)],
                        id_sb[:, :],
                    )
                    nc.vector.tensor_copy(
                        out=v_sb[:, 4 * tb + s, :], in_=vt_tp[:, 0:128]
                    )

                rot_ps = {}
                for h in range(NH_C):
                    rot_ps[h] = nar("rot_ps")
                    nc.tensor.matmul(
                        rot_ps[h], lhsT=rt_sb[:, :], rhs=qraws[h],
                        start=True, stop=True,
                    )
                    if h % 2 == 1:
                        emit_oproj_unit("v" if h == 1 else "s")
                    nc.vector.tensor_mul(
                        qf[:, h, :], qraws[h], cos_sb[:, ts0: ts0 + 512]
                    )
                    rtmp = work.tile([128, 512], DT, name="rtmp", bufs=2)
                    nc.vector.tensor_mul(
                        rtmp, rot_ps[h], sin_sb[:, ts0: ts0 + 512]
                    )
                    nc.vector.tensor_add(qf[:, h, :], qf[:, h, :], rtmp)
                emit_oproj_unit("v")
                stt["qf"] = qf

            def emit_attn(tb, stt):
                # ============ stage 4: attention for q-block jq == tb
                # Heads in pairs; both heads' scores live in one wide PSUM
                # tile so exp is a single [128, 2, N] ACT op.  PV/denominator
                # of k-tile i-1 issue while ACT computes exp of tile i.
                qf = stt["qf"]
                ot_sb = {}
                imax = 4 * tb + 3
                for hp in range(NH_C // 2):
                    heads = (2 * hp, 2 * hp + 1)
                    ot_ps = {h: nar(f"ot_ps{h}") for h in heads}
                    if den_pack:
                        den_ps = nar("den_ps")
                        den_ap = {heads[0]: den_ps[0:1, :],
                                  heads[1]: den_ps[32:33, :]}
                        den_tp = {heads[0]: None, heads[1]: (0, 32)}
                        den_start = {heads[0]: lambda i: i == 0,
                                     heads[1]: lambda i: False}
                    else:
                        # separate banks per head, but head 1's row sits at
                        # partition 32 via tile_position so the two den
                        # matmuls target different PE column groups (they can
                        # overlap in the array).
                        dps = {h: nar(f"den_ps{h}") for h in heads}
                        if den_tile:
                            den_ap = {heads[0]: dps[heads[0]][0:1, :],
                                      heads[1]: dps[heads[1]][32:33, :]}
                            den_tp = {heads[0]: None, heads[1]: (0, 32)}
                        else:
                            den_ap = {h: dps[h][0:1, :] for h in heads}
                            den_tp = {h: None for h in heads}
                        den_start = {h: (lambda i: i == 0) for h in heads}

                    def emit_pv_den(i, pt, c0):
                        # dens first (reciprocal chain starts earlier) and
                        # adjacent (they share the ones16 stationary operand),
                        # then both pvs (they share the v tile).
                        first, last = i == 0, i == imax
                        for j, h in enumerate(heads):
                            kw = {}
                            if den_tp[h] is not None:
                                kw["tile_position"] = den_tp[h]
                            nc.tensor.matmul(
                                den_ap[h][:, c0:],
                                lhsT=ones16[:, 0:1],
                                rhs=pt[:, j, c0:],
                                start=den_start[h](i), stop=last,
                                skip_group_check=True,
                                **kw,
                            )
                        for j, h in enumerate(heads):
                            nc.tensor.matmul(
                                ot_ps[h][:, c0:],
                                lhsT=v_sb[:, i, :],
                                rhs=pt[:, j, c0:],
                                start=first, stop=last,
                            )

                    prev = None
                    for i in range(imax + 1):
                        c0 = 128 * max(0, i - 4 * tb)
                        diag = i >= 4 * tb
                        st = wide("st")
                        for j, h in enumerate(heads):
                            nc.tensor.matmul(
                                st[:, j, c0:],
                                lhsT=kt_sb[:, 128 * i: 128 * (i + 1)],
                                rhs=qf[:, h, c0:],
                                start=True, stop=True,
                            )
                        pt = ptp.tile([128, 2, 512], DT, name="pt")
                        nc.scalar.activation(
                            out=pt[:, :, c0:], in_=st[:, :, c0:], func=exp_f,
                            scale=INV_SQRT_HS,
                        )
                        if diag:
                            for j in range(2):
                                if gp_mask:
                                    # zero strictly-lower triangle (tk > tq)
                                    nc.gpsimd.affine_select(
                                        out=pt[:, j, c0: c0 + 128],
                                        in_=pt[:, j, c0: c0 + 128],
                                        compare_op=mybir.AluOpType.is_ge,
                                        fill=0.0,
                                        base=0,
                                        pattern=[[1, 128]],
                                        channel_multiplier=-1,
                                    )
                                else:
                                    nc.vector.tensor_mul(
                                        pt[:, j, c0: c0 + 128],
                                        pt[:, j, c0: c0 + 128],
                                        mask_sb,
                                    )
                        if prev is not None:
                            emit_pv_den(*prev)
                        prev = (i, pt, c0)
                    emit_pv_den(*prev)
                    if pair_oproj:
                        emit_oproj_unit("s" if hp == 0 else "v")

                    # normalize each head's O^T by its softmax denominator:
                    # 1/den straight from PSUM (reciprocal_approx_fast) frees
                    # the den banks at once; ACT evacuates O^T (freeing the ot
                    # banks for the next pair); PE broadcasts the reciprocal
                    # across partitions and DVE multiplies it in-place,
                    # reading the broadcast directly from PSUM.
                    denrec = {}
                    for j, h in enumerate(heads):
                        draw = drow.tile([1, 512], f32, name="draw")
                        if recip_psum:
                            nc.vector.reciprocal_approx_fast(
                                out=draw, in_=den_ap[h]
                            )
                        else:
                            dcp = drow.tile([1, 512], f32, name="dcp")
                            nc.scalar.copy(out=dcp, in_=den_ap[h])
                            nc.vector.reciprocal_approx_fast(
                                out=draw, in_=dcp
                            )
                        denrec[h] = drow.tile([1, 512], f32, name="denrec")
                        nc.vector.tensor_copy(
                            out=denrec[h][:, :].bitcast(f32r), in_=draw
                        )
                    for j, h in enumerate(heads):
                        osb = otp.tile([128, 512], DT, name="osb")
                        nc.scalar.copy(out=osb, in_=ot_ps[h])
                        ot_sb[h] = osb
                    for j, h in enumerate(heads):
                        bc_ps = nar("bc_ps")
                        nc.tensor.matmul(
                            bc_ps,
                            lhsT=onesf[:, :].bitcast(f32r),
                            rhs=denrec[h][:, :].bitcast(f32r),
                            start=True, stop=True,
                        )
                        nc.vector.tensor_mul(ot_sb[h], ot_sb[h], bc_ps)

                for s in range(4):
                    oproj_units.append((tb, s, ot_sb))

            # ============ pipelined schedule: projections of block tb+1 are
            # emitted before attention of block tb, so the PE's projection
            # stream covers the serial evac/rope chains on ACT/DVE, and the
            # rope of tb+1 lands after attention of tb where the previous
            # block's o_proj units fill the rot-matmul waits.
            # warm-up: dependency-free matmuls keep the PE busy while the
            # first DMA chunks land, so the HAM clock-gate is already at
            # 8/8 when the real matmuls start (cold matmuls run at 1.2GHz).
            warm_sb = consts.tile([128, 64], DT, name="warm_sb")
            nc.vector.memset(warm_sb, 1.0)
            warm_ps = nar("warm_ps")
            for _ in range(56):
                nc.tensor.matmul(
                    warm_ps[0:64, 0:64], lhsT=warm_sb[:, 0:64],
                    rhs=warm_sb[:, 0:64], start=True, stop=True,
                    skip_group_check=True,
                )

            states = {}
            states[0] = emit_proj(0)
            emit_evac(0, states[0])
            emit_rope(0, states[0])
            for tb in range(tb_n):
                if tb + 1 < tb_n:
                    states[tb + 1] = emit_proj(tb + 1)
                    emit_evac(tb + 1, states[tb + 1])
                emit_attn(tb, states[tb])
                if tb + 1 < tb_n:
                    emit_rope(tb + 1, states[tb + 1])
                    states.pop(tb)

            while oproj_units:
                emit_oproj_unit("s", split_dma=True)
                emit_oproj_unit("v", split_dma=True)

    nc.compile()
    return nc


def shard_inputs(x, cos, sin, Wq, bq, Wkv, bkv, Wo, t=T, dt16="bf16"):
    """Build the 8 per-core input maps (core c -> batch c//4, group c%4)."""
    import ml_dtypes

    DT = {"bf16": ml_dtypes.bfloat16, "fp16": np.float16}[dt16]
    f32 = np.float32
    hs = HS
    rot = np.zeros((hs, hs), f32)
    for i in range(hs // 2):
        rot[i, i + hs // 2] = -1.0
        rot[i + hs // 2, i] = 1.0
    r_t = np.ascontiguousarray(rot.T).astype(DT)
    mask_ut = np.triu(np.ones((128, 128), f32)).astype(DT)
    ident = np.eye(128, dtype=f32)
    cos_t = np.ascontiguousarray(cos.T).astype(DT)
    sin_t = np.ascontiguousarray(sin.T).astype(DT)

    xts = [np.ascontiguousarray(x[b].T).astype(DT) for b in range(x.shape[0])]
    per_g = []
    for g in range(4):
        per_g.append(
            dict(
                wq_t=np.ascontiguousarray(
                    Wq[512 * g: 512 * g + 512].T).astype(DT),
                b_q=np.ascontiguousarray(
                    bq[512 * g: 512 * g + 512].reshape(4, 128).T.astype(f32)
                ),
                wk_t=np.ascontiguousarray(
                    Wkv[128 * g: 128 * g + 128].T).astype(DT),
                b_k=np.ascontiguousarray(
                    bkv[128 * g: 128 * g + 128].reshape(128, 1).astype(f32)
                ),
                wv_t=np.ascontiguousarray(
                    Wkv[512 + 128 * g: 512 + 128 * g + 128].T).astype(DT),
                b_v=np.ascontiguousarray(
                    bkv[512 + 128 * g: 512 + 128 * g + 128]
                    .reshape(128, 1)
                    .astype(f32)
                ),
                wo_t=np.ascontiguousarray(
                    Wo[:, 512 * g: 512 * g + 512].T).astype(DT),
            )
        )

    in_maps = []
    for c in range(4 * x.shape[0]):
        b, g = c // 4, c % 4
        m = dict(per_g[g])
        m.update(
            x_t=xts[b], cos_t=cos_t, sin_t=sin_t,
            r_t=r_t, mask_ut=mask_ut, ident=ident,
        )
        in_maps.append(m)
    return in_maps


def run_on_hw(in_maps, t=T, trace=False, **flags):
    from concourse.bass_utils import run_bass_kernel_spmd

    key = (t, tuple(sorted(flags.items())))
    if key not in _NC_CACHE:
        _NC_CACHE[key] = build_nc(t, **flags)
    nc = _NC_CACHE[key]
    res = run_bass_kernel_spmd(
        nc, in_maps, core_ids=list(range(len(in_maps))), trace=trace
    )
    return res


def kernel(x, cos, sin, Wq, bq, Wkv, bkv, Wo):
    x = np.asarray(x)
    in_maps = shard_inputs(
        x, np.asarray(cos), np.asarray(sin), np.asarray(Wq), np.asarray(bq),
        np.asarray(Wkv), np.asarray(bkv), np.asarray(Wo),
    )
    res = run_on_hw(in_maps, t=T, trace=False)
    out = np.zeros((B, T, D), np.float32)
    for c, rmap in enumerate(res.results):
        out[c // 4] += rmap["out"]
    return out


# revision 48
# speedup vs baseline: 1.0064x; 1.0064x over previous
"""Causal group-query attention on 8 Trainium2 NeuronCores (bf16 edition).

Sharding: core c -> (batch b = c // 4, kv-group g = c % 4).
Each core owns batch element b, q-heads [4g, 4g+4) and kv-group g (n_rep = 4,
so those 4 q-heads attend to exactly kv-group g's k/v).  Every core computes
its partial o_proj output (contracting head-concat columns [512g, 512g+512)),
and the host sums the 4 partials per batch element (the "all-reduce after
o_proj" done host-side since we return full outputs anyway).

v2 vs the fp32r baseline (394us):
  * all matmuls in bf16: moving operand streams 2 elem/cycle (vs 1 for
    fp32r) and FWL kicks in for the 128-col weight loads -> ~2x PE.
  * PSUM laid out as 2 wide [128,2,512] tiles + 4 narrow [128,512] tiles
    (8 banks total).  Phase A: qt01/qt23 in the wide tiles, kt/vt narrow.
    Phase B: the wide tiles double-buffer the head-pair score banks, so
    one ACT instruction computes exp for BOTH heads of a pair ([128,2,N]
    3-D AP), halving ACT's 352-cycle per-instruction overhead.
  * both heads' softmax denominators accumulate into ONE narrow bank at
    partitions 0/32 (tile_position=(0,32j)); only the first matmul of the
    pair uses start=True, so the whole-bank has_written clear happens
    exactly once.
  * reciprocal via reciprocal_approx_fast ([1,512], ~5x faster than
    nc.vector.reciprocal) straight from PSUM; the raw->recip->PE
    broadcast->DVE multiply chain never touches ACT.
  * q/k/v bias-adds evacuate PSUM on DVE (tensor_scalar add) instead of
    ACT activations; causal masking runs on the otherwise-idle GpSimd.
  * o_proj of block tb-1 is emitted interleaved with block tb's RoPE so
    the PE always has ready work while ACT/DVE chew the serial chains.
"""

import math

import numpy as np

B, T, D = 2, 2048, 2048
N_HEAD, N_GROUP = 16, 4
HS = D // N_HEAD  # 128
N_REP = N_HEAD // N_GROUP  # 4
NH_C = N_HEAD // N_GROUP  # heads per core = 4
INV_SQRT_HS = 1.0 / math.sqrt(HS)

_NC_CACHE: dict = {}


def build_nc(t=T, dt16="bf16", den_pack=False, recip_psum=True, gp_mask=True,
             pair_oproj=True, den_tile=False):
    """Build and compile the per-core Bass program. Returns the compiled nc."""
    import concourse.bass as bass  # noqa: F401
    import concourse.mybir as mybir
    import concourse.tile as tile
    from concourse import bacc

    f32 = mybir.dt.float32
    f32r = mybir.dt.float32r
    DT = {"bf16": mybir.dt.bfloat16, "fp16": mybir.dt.float16}[dt16]
    exp_f = mybir.ActivationFunctionType.Exp
    ident_f = mybir.ActivationFunctionType.Identity

    # pair_oproj needs the nar-ring slot den_pack frees up; without it the
    # ring WAR chain (op unit -> ot read -> bc matmul) can deadlock.
    pair_oproj = pair_oproj and den_pack

    nd = D // 128  # d-tiles (contraction) = 16
    tb_n = t // 512  # 512-wide t blocks
    nk = t // 128  # 128-wide k tiles

    nc = bacc.Bacc("TRN2", target_bir_lowering=False, debug=False)

    xd = nc.dram_tensor("x_t", [D, t], DT, kind="ExternalInput")
    wqd = nc.dram_tensor("wq_t", [D, NH_C * HS], DT, kind="ExternalInput")
    wkd = nc.dram_tensor("wk_t", [D, HS], DT, kind="ExternalInput")
    wvd = nc.dram_tensor("wv_t", [D, HS], DT, kind="ExternalInput")
    wod = nc.dram_tensor("wo_t", [NH_C * HS, D], DT, kind="ExternalInput")
    cosd = nc.dram_tensor("cos_t", [HS, t], DT, kind="ExternalInput")
    sind = nc.dram_tensor("sin_t", [HS, t], DT, kind="ExternalInput")
    bqd = nc.dram_tensor("b_q", [HS, NH_C], f32, kind="ExternalInput")
    bkd = nc.dram_tensor("b_k", [HS, 1], f32, kind="ExternalInput")
    bvd = nc.dram_tensor("b_v", [HS, 1], f32, kind="ExternalInput")
    rtd = nc.dram_tensor("r_t", [HS, HS], DT, kind="ExternalInput")
    maskd = nc.dram_tensor("mask_ut", [128, 128], DT, kind="ExternalInput")
    identd = nc.dram_tensor("ident", [128, 128], f32, kind="ExternalInput")
    outd = nc.dram_tensor("out", [t, D], f32, kind="ExternalOutput")

    with tile.TileContext(nc) as tc:
        with (
            tc.tile_pool(name="consts", bufs=1) as consts,
            tc.tile_pool(name="wpool", bufs=1) as wpool,
            tc.tile_pool(name="resid", bufs=1) as resid,
            tc.tile_pool(name="xin", bufs=10) as xin,
            tc.tile_pool(name="work", bufs=4) as work,
            tc.tile_pool(name="vwork", bufs=2) as vwork,
            tc.tile_pool(name="drow", bufs=4) as drow,
            tc.tile_pool(name="qfp", bufs=2) as qfp,
            tc.tile_pool(name="ptp", bufs=4) as ptp,
            tc.tile_pool(name="otp", bufs=8) as otp,
            tc.tile_pool(name="bcp", bufs=2) as bcp,
            tc.tile_pool(name="outp", bufs=4) as outp,
            tc.tile_pool(name="psw", bufs=2, space="PSUM") as psw,
            tc.tile_pool(name="psn", bufs=4, space="PSUM") as psn,
        ):
            def wide(name):
                return psw.tile([128, 2, 512], f32, tag="wide", name=name)

            def nar(name):
                return psn.tile([128, 512], f32, tag="nar", name=name)

            # ---- constants / weights (loaded once) ----
            cos_sb = consts.tile([128, t], DT, name="cos_sb")
            sin_sb = consts.tile([128, t], DT, name="sin_sb")
            rt_sb = consts.tile([128, 128], DT, name="rt_sb")
            mask_sb = consts.tile([128, 128], DT, name="mask_sb")
            id_sb = consts.tile([128, 128], f32, name="id_sb")
            ones16 = consts.tile([128, 1], DT, name="ones16")
            onesf_raw = consts.tile([1, 128], f32, name="onesf_raw")
            onesf = consts.tile([1, 128], f32, name="onesf")
            bq_sb = consts.tile([128, NH_C], f32, name="bq_sb")
            bk_sb = consts.tile([128, 1], f32, name="bk_sb")
            bv_sb = consts.tile([128, 1], f32, name="bv_sb")
            wq_sb = wpool.tile([128, nd, NH_C * HS], DT, name="wq_sb")
            wk_sb = wpool.tile([128, nd, HS], DT, name="wk_sb")
            wv_sb = wpool.tile([128, nd, HS], DT, name="wv_sb")
            wo_sb = wpool.tile([128, NH_C, D], DT, name="wo_sb")
            wq_re = wqd[:, :].rearrange("(n p) m -> p n m", p=128)
            wk_re = wkd[:, :].rearrange("(n p) m -> p n m", p=128)
            wv_re = wvd[:, :].rearrange("(n p) m -> p n m", p=128)

            # resident K^T [hs, t] and V [t(128-tiles), hs]
            kt_sb = resid.tile([128, t], DT, name="kt_sb")
            v_sb = resid.tile([128, nk, HS], DT, name="v_sb")

            x_re = xd[:, :].rearrange("(n p) t -> p n t", p=128)

            oproj_units = []  # deferred (tb, s, ot_sb) work units
            oproj_q = [0]  # alternate output-DMA queue (gpsimd / sync)

            def emit_oproj_unit(evac_eng, split_dma=False):
                # one unit = a full 128-row output block: 4 db sub-blocks of
                # 4 head-accumulated matmuls each, evacuated into one wide
                # [128, 2048] tile and shipped with a single DMA.  With
                # split_dma (tail units) each db half ships separately so the
                # transfer overlaps the remaining evacuations.
                if not oproj_units:
                    return
                tb, s, ot_sb = oproj_units.pop(0)
                ts0 = tb * 512
                ob = outp.tile([128, D], f32, name="ob")
                # db-pairs with h outer: each head's stationary osb slice is
                # loaded once per pair of db sub-blocks instead of per block.
                for dbp in range(2):
                    op_ps = {db: nar("op_ps") for db in (2 * dbp, 2 * dbp + 1)}
                    for h in range(NH_C):
                        for db in (2 * dbp, 2 * dbp + 1):
                            nc.tensor.matmul(
                                op_ps[db],
                                lhsT=ot_sb[h][:, 128 * s: 128 * (s + 1)],
                                rhs=wo_sb[:, h, 512 * db: 512 * (db + 1)],
                                start=h == 0, stop=h == NH_C - 1,
                            )
                    for db in (2 * dbp, 2 * dbp + 1):
                        ob_sl = ob[:, 512 * db: 512 * (db + 1)]
                        if (db % 2 == 0) == (evac_eng == "v"):
                            nc.vector.tensor_copy(out=ob_sl, in_=op_ps[db])
                        else:
                            nc.scalar.copy(out=ob_sl, in_=op_ps[db])
                    if split_dma:
                        eng = nc.gpsimd if oproj_q[0] % 2 == 0 else nc.sync
                        oproj_q[0] += 1
                        eng.dma_start(
                            out=outd[
                                ts0 + 128 * s: ts0 + 128 * (s + 1),
                                1024 * dbp: 1024 * (dbp + 1),
                            ],
                            in_=ob[:, 1024 * dbp: 1024 * (dbp + 1)],
                        )
                if not split_dma:
                    eng = nc.gpsimd if oproj_q[0] % 2 == 0 else nc.sync
                    oproj_q[0] += 1
                    eng.dma_start(
                        out=outd[ts0 + 128 * s: ts0 + 128 * (s + 1), :],
                        in_=ob,
                    )

            def emit_proj(tb):
                # ============ stage 1: q/k/v projection matmuls for block tb
                ts0 = tb * 512
                qt01 = wide("qt01")
                qt23 = wide("qt23")
                kt_ps = nar("kt_ps")
                vt_ps = nar("vt_ps")
                xts_tb = []
                for chunk in range(nd // 2):
                    c2 = 2 * chunk
                    xt = xin.tile([128, 2, 512], DT, name="xt")
                    xts_tb.append(xt)
                    nc.sync.dma_start(
                        out=xt, in_=x_re[:, c2: c2 + 2, ts0: ts0 + 512]
                    )
                    if tb == 0:
                        nc.sync.dma_start(
                            out=wq_sb[:, c2: c2 + 2, :],
                            in_=wq_re[:, c2: c2 + 2, :],
                        )
                        nc.sync.dma_start(
                            out=wk_sb[:, c2: c2 + 2, :],
                            in_=wk_re[:, c2: c2 + 2, :],
                        )
                        nc.sync.dma_start(
                            out=wv_sb[:, c2: c2 + 2, :],
                            in_=wv_re[:, c2: c2 + 2, :],
                        )
                        if chunk == 0:
                            nc.sync.dma_start(out=bq_sb, in_=bqd[:, :])
                            nc.sync.dma_start(out=bk_sb, in_=bkd[:, :])
                            nc.sync.dma_start(out=bv_sb, in_=bvd[:, :])
                            nc.vector.memset(ones16, 1.0)
                            nc.vector.memset(onesf_raw, 1.0)
                            # fp32r matmul operands must come from a
                            # producer with f32r-typed output (walrus rule)
                            nc.vector.tensor_copy(
                                out=onesf[:, :].bitcast(f32r), in_=onesf_raw
                            )
                    for j in range(2):
                        dt = c2 + j
                        first, last = dt == 0, dt == nd - 1
                        for h in range(2):
                            nc.tensor.matmul(
                                qt01[:, h, :],
                                lhsT=wq_sb[:, dt, h * HS: (h + 1) * HS],
                                rhs=xt[:, j, :],
                                start=first, stop=last,
                            )
                        for h in range(2):
                            nc.tensor.matmul(
                                qt23[:, h, :],
                                lhsT=wq_sb[:, dt, (2 + h) * HS: (3 + h) * HS],
                                rhs=xt[:, j, :],
                                start=first, stop=last,
                            )
                        nc.tensor.matmul(
                            kt_ps, lhsT=wk_sb[:, dt, :], rhs=xt[:, j, :],
                            start=first, stop=last,
                        )
                        nc.tensor.matmul(
                            vt_ps, lhsT=wv_sb[:, dt, :], rhs=xt[:, j, :],
                            start=first, stop=last,
                        )

                if tb == 0:
                    # one-time loads go on the gpsimd queue so the sync queue
                    # stays dedicated to the xt stream.  Gate the queue on
                    # chunk 3's arrival so these bulk loads don't steal HBM
                    # bandwidth from the critical first x/w chunks.
                    gate = drow.tile([1, 1], DT, name="gate")
                    nc.gpsimd.tensor_copy(
                        out=gate, in_=xts_tb[3][0:1, 0, 0:1]
                    )
                    nc.gpsimd.dma_start(out=rt_sb, in_=rtd[:, :])
                    nc.gpsimd.dma_start(out=id_sb, in_=identd[:, :])
                    nc.gpsimd.dma_start(out=mask_sb, in_=maskd[:, :])
                    nc.gpsimd.dma_start(out=cos_sb, in_=cosd[:, :])
                    nc.gpsimd.dma_start(out=sin_sb, in_=sind[:, :])
                    wo_re = wod[:, :].rearrange("(h p) m -> p h m", p=128)
                    for h in range(NH_C):
                        nc.gpsimd.dma_start(
                            out=wo_sb[:, h: h + 1, :],
                            in_=wo_re[:, h: h + 1, :],
                        )
                return dict(qt01=qt01, qt23=qt23, kt_ps=kt_ps, vt_ps=vt_ps)

            def emit_evac(tb, stt):
                # ============ stage 2: bias-add PSUM evacuations.  k/v on
                # DVE (the k-rope chain is the critical path), q on ACT so
                # the two chains overlap.  Frees all of stage 1's banks.
                kraw = work.tile([128, 512], DT, name="kraw", bufs=2)
                nc.vector.tensor_scalar_add(
                    out=kraw, in0=stt["kt_ps"], scalar1=bk_sb[:, 0:1]
                )
                vraw = vwork.tile([128, 512], f32, name="vraw")
                nc.vector.tensor_scalar_add(
                    out=vraw, in0=stt["vt_ps"], scalar1=bv_sb[:, 0:1]
                )
                qraws = []
                for h in range(NH_C):
                    qraw = work.tile([128, 512], DT, name="qraw")
                    src = (stt["qt01"][:, h, :] if h < 2
                           else stt["qt23"][:, h - 2, :])
                    nc.scalar.activation(
                        out=qraw, in_=src, func=ident_f,
                        bias=bq_sb[:, h: h + 1], scale=1.0,
                    )
                    qraws.append(qraw)
                stt.update(kraw=kraw, vraw=vraw, qraws=qraws)

            def emit_rope(tb, stt):
                # ============ stage 3: RoPE (k then q) + V transpose.
                # o_proj units of the previous block keep the PE fed while
                # ACT/DVE chew the serial chains.
                ts0 = tb * 512
                kraw, vraw, qraws = stt["kraw"], stt["vraw"], stt["qraws"]
                qf = qfp.tile([128, NH_C, 512], DT, name="qf")
                rot_k = nar("rot_ps")
                nc.tensor.matmul(
                    rot_k, lhsT=rt_sb[:, :], rhs=kraw, start=True, stop=True
                )
                emit_oproj_unit("s")
                nc.vector.tensor_mul(
                    kt_sb[:, ts0: ts0 + 512], kraw, cos_sb[:, ts0: ts0 + 512]
                )
                rtmp = work.tile([128, 512], DT, name="rtmp", bufs=2)
                nc.vector.tensor_mul(rtmp, rot_k, sin_sb[:, ts0: ts0 + 512])
                nc.vector.tensor_add(
                    kt_sb[:, ts0: ts0 + 512], kt_sb[:, ts0: ts0 + 512], rtmp
                )

                # v: transpose [hs, t] -> [t, hs] tiles
                for s in range(4):
                    vt_tp = nar("vt_tp")
                    nc.tensor.transpose(
                        vt_tp[:, 0:128], vraw[:, 128 * s: 128 * (s + 1)],
                        id_sb[:, :],
                    )
                    nc.vector.tensor_copy(
                        out=v_sb[:, 4 * tb + s, :], in_=vt_tp[:, 0:128]
                    )

                rot_ps = {}
                for h in range(NH_C):
                    rot_ps[h] = nar("rot_ps")
                    nc.tensor.matmul(
                        rot_ps[h], lhsT=rt_sb[:, :], rhs=qraws[h],
                        start=True, stop=True,
                    )
                    if h % 2 == 1:
                        emit_oproj_unit("v" if h == 1 else "s")
                    nc.vector.tensor_mul(
                        qf[:, h, :], qraws[h], cos_sb[:, ts0: ts0 + 512]
                    )
                    rtmp = work.tile([128, 512], DT, name="rtmp", bufs=2)
                    nc.vector.tensor_mul(
                        rtmp, rot_ps[h], sin_sb[:, ts0: ts0 + 512]
                    )
                    nc.vector.tensor_add(qf[:, h, :], qf[:, h, :], rtmp)
                emit_oproj_unit("v")
                stt["qf"] = qf

            def emit_attn(tb, stt):
                # ============ stage 4: attention for q-block jq == tb
                # Heads in pairs; both heads' scores live in one wide PSUM
                # tile so exp is a single [128, 2, N] ACT op.  PV/denominator
                # of k-tile i-1 issue while ACT computes exp of tile i.
                qf = stt["qf"]
                ot_sb = {}
                imax = 4 * tb + 3
                for hp in range(NH_C // 2):
                    heads = (2 * hp, 2 * hp + 1)
                    ot_ps = {h: nar(f"ot_ps{h}") for h in heads}
                    if den_pack:
                        den_ps = nar("den_ps")
                        den_ap = {heads[0]: den_ps[0:1, :],
                                  heads[1]: den_ps[32:33, :]}
                        den_tp = {heads[0]: None, heads[1]: (0, 32)}
                        den_start = {heads[0]: lambda i: i == 0,
                                     heads[1]: lambda i: False}
                    else:
                        # separate banks per head, but head 1's row sits at
                        # partition 32 via tile_position so the two den
                        # matmuls target different PE column groups (they can
                        # overlap in the array).
                        dps = {h: nar(f"den_ps{h}") for h in heads}
                        if den_tile:
                            den_ap = {heads[0]: dps[heads[0]][0:1, :],
                                      heads[1]: dps[heads[1]][32:33, :]}
                            den_tp = {heads[0]: None, heads[1]: (0, 32)}
                        else:
                            den_ap = {h: dps[h][0:1, :] for h in heads}
                            den_tp = {h: None for h in heads}
                        den_start = {h: (lambda i: i == 0) for h in heads}

                    def emit_pv_den(i, pt, c0):
                        # dens first (reciprocal chain starts earlier) and
                        # adjacent (they share the ones16 stationary operand),
                        # then both pvs (they share the v tile).
                        first, last = i == 0, i == imax
                        for j, h in enumerate(heads):
                            kw = {}
                            if den_tp[h] is not None:
                                kw["tile_position"] = den_tp[h]
                            nc.tensor.matmul(
                                den_ap[h][:, c0:],
                                lhsT=ones16[:, 0:1],
                                rhs=pt[:, j, c0:],
                                start=den_start[h](i), stop=last,
                                skip_group_check=True,
                                **kw,
                            )
                        for j, h in enumerate(heads):
                            nc.tensor.matmul(
                                ot_ps[h][:, c0:],
                                lhsT=v_sb[:, i, :],
                                rhs=pt[:, j, c0:],
                                start=first, stop=last,
                            )

                    prev = None
                    for i in range(imax + 1):
                        c0 = 128 * max(0, i - 4 * tb)
                        diag = i >= 4 * tb
                        st = wide("st")
                        for j, h in enumerate(heads):
                            nc.tensor.matmul(
                                st[:, j, c0:],
                                lhsT=kt_sb[:, 128 * i: 128 * (i + 1)],
                                rhs=qf[:, h, c0:],
                                start=True, stop=True,
                            )
                        pt = ptp.tile([128, 2, 512], DT, name="pt")
                        nc.scalar.activation(
                            out=pt[:, :, c0:], in_=st[:, :, c0:], func=exp_f,
                            scale=INV_SQRT_HS,
                        )
                        if diag:
                            for j in range(2):
                                if gp_mask:
                                    # zero strictly-lower triangle (tk > tq)
                                    nc.gpsimd.affine_select(
                                        out=pt[:, j, c0: c0 + 128],
                                        in_=pt[:, j, c0: c0 + 128],
                                        compare_op=mybir.AluOpType.is_ge,
                                        fill=0.0,
                                        base=0,
                                        pattern=[[1, 128]],
                                        channel_multiplier=-1,
                                    )
                                else:
                                    nc.vector.tensor_mul(
                                        pt[:, j, c0: c0 + 128],
                                        pt[:, j, c0: c0 + 128],
                                        mask_sb,
                                    )
                        if prev is not None:
                            emit_pv_den(*prev)
                        prev = (i, pt, c0)
                    emit_pv_den(*prev)
                    if pair_oproj:
                        emit_oproj_unit("s" if hp == 0 else "v")

                    # normalize each head's O^T by its softmax denominator:
                    # 1/den straight from PSUM (reciprocal_approx_fast) frees
                    # the den banks at once; ACT evacuates O^T (freeing the ot
                    # banks for the next pair); PE broadcasts the reciprocal
                    # across partitions and DVE multiplies it in-place,
                    # reading the broadcast directly from PSUM.
                    denrec = {}
                    for j, h in enumerate(heads):
                        draw = drow.tile([1, 512], f32, name="draw")
                        if recip_psum:
                            nc.vector.reciprocal_approx_fast(
                                out=draw, in_=den_ap[h]
                            )
                        else:
                            dcp = drow.tile([1, 512], f32, name="dcp")
                            nc.scalar.copy(out=dcp, in_=den_ap[h])
                            nc.vector.reciprocal_approx_fast(
                                out=draw, in_=dcp
                            )
                        denrec[h] = drow.tile([1, 512], f32, name="denrec")
                        nc.vector.tensor_copy(
                            out=denrec[h][:, :].bitcast(f32r), in_=draw
                        )
                    for j, h in enumerate(heads):
                        osb = otp.tile([128, 512], DT, name="osb")
                        nc.scalar.copy(out=osb, in_=ot_ps[h])
                        ot_sb[h] = osb
                    for j, h in enumerate(heads):
                        bc_ps = nar("bc_ps")
                        nc.tensor.matmul(
                            bc_ps,
                            lhsT=onesf[:, :].bitcast(f32r),
                            rhs=denrec[h][:, :].bitcast(f32r),
                            start=True, stop=True,
                        )
                        nc.vector.tensor_mul(ot_sb[h], ot_sb[h], bc_ps)

                for s in range(4):
                    oproj_units.append((tb, s, ot_sb))

            # ============ pipelined schedule: projections of block tb+1 are
            # emitted before attention of block tb, so the PE's projection
            # stream covers the serial evac/rope chains on ACT/DVE, and the
            # rope of tb+1 lands after attention of tb where the previous
            # block's o_proj units fill the rot-matmul waits.
            # warm-up: dependency-free matmuls keep the PE busy while the
            # first DMA chunks land, so the HAM clock-gate is already at
            # 8/8 when the real matmuls start (cold matmuls run at 1.2GHz).
            warm_sb = consts.tile([128, 64], DT, name="warm_sb")
            nc.vector.memset(warm_sb, 1.0)
            warm_ps = nar("warm_ps")
            for _ in range(20):
                nc.tensor.matmul(
                    warm_ps[0:64, 0:64], lhsT=warm_sb[:, 0:64],
                    rhs=warm_sb[:, 0:64], start=True, stop=True,
                    skip_group_check=True,
                )

            states = {}
            states[0] = emit_proj(0)
            emit_evac(0, states[0])
            emit_rope(0, states[0])
            for tb in range(tb_n):
                if tb + 1 < tb_n:
                    states[tb + 1] = emit_proj(tb + 1)
                    emit_evac(tb + 1, states[tb + 1])
                emit_attn(tb, states[tb])
                if tb + 1 < tb_n:
                    emit_rope(tb + 1, states[tb + 1])
                    states.pop(tb)

            while oproj_units:
                emit_oproj_unit("s", split_dma=True)
                emit_oproj_unit("v", split_dma=True)

    nc.compile()
    return nc


def shard_inputs(x, cos, sin, Wq, bq, Wkv, bkv, Wo, t=T, dt16="bf16"):
    """Build the 8 per-core input maps (core c -> batch c//4, group c%4)."""
    import ml_dtypes

    DT = {"bf16": ml_dtypes.bfloat16, "fp16": np.float16}[dt16]
    f32 = np.float32
    hs = HS
    rot = np.zeros((hs, hs), f32)
    for i in range(hs // 2):
        rot[i, i + hs // 2] = -1.0
        rot[i + hs // 2, i] = 1.0
    r_t = np.ascontiguousarray(rot.T).astype(DT)
    mask_ut = np.triu(np.ones((128, 128), f32)).astype(DT)
    ident = np.eye(128, dtype=f32)
    cos_t = np.ascontiguousarray(cos.T).astype(DT)
    sin_t = np.ascontiguousarray(sin.T).astype(DT)

    xts = [np.ascontiguousarray(x[b].T).astype(DT) for b in range(x.shape[0])]
    per_g = []
    for g in range(4):
        per_g.append(
            dict(
                wq_t=np.ascontiguousarray(
                    Wq[512 * g: 512 * g + 512].T).astype(DT),
                b_q=np.ascontiguousarray(
                    bq[512 * g: 512 * g + 512].reshape(4, 128).T.astype(f32)
                ),
                wk_t=np.ascontiguousarray(
                    Wkv[128 * g: 128 * g + 128].T).astype(DT),
                b_k=np.ascontiguousarray(
                    bkv[128 * g: 128 * g + 128].reshape(128, 1).astype(f32)
                ),
                wv_t=np.ascontiguousarray(
                    Wkv[512 + 128 * g: 512 + 128 * g + 128].T).astype(DT),
                b_v=np.ascontiguousarray(
                    bkv[512 + 128 * g: 512 + 128 * g + 128]
                    .reshape(128, 1)
                    .astype(f32)
                ),
                wo_t=np.ascontiguousarray(
                    Wo[:, 512 * g: 512 * g + 512].T).astype(DT),
            )
        )

    in_maps = []
    for c in range(4 * x.shape[0]):
        b, g = c // 4, c % 4
        m = dict(per_g[g])
        m.update(
            x_t=xts[b], cos_t=cos_t, sin_t=sin_t,
            r_t=r_t, mask_ut=mask_ut, ident=ident,
        )
        in_maps.append(m)
    return in_maps


def run_on_hw(in_maps, t=T, trace=False, **flags):
    from concourse.bass_utils import run_bass_kernel_spmd

    key = (t, tuple(sorted(flags.items())))
    if key not in _NC_CACHE:
        _NC_CACHE[key] = build_nc(t, **flags)
    nc = _NC_CACHE[key]
    res = run_bass_kernel_spmd(
        nc, in_maps, core_ids=list(range(len(in_maps))), trace=trace
    )
    return res


def kernel(x, cos, sin, Wq, bq, Wkv, bkv, Wo):
    x = np.asarray(x)
    in_maps = shard_inputs(
        x, np.asarray(cos), np.asarray(sin), np.asarray(Wq), np.asarray(bq),
        np.asarray(Wkv), np.asarray(bkv), np.asarray(Wo),
    )
    res = run_on_hw(in_maps, t=T, trace=False)
    out = np.zeros((B, T, D), np.float32)
    for c, rmap in enumerate(res.results):
        out[c // 4] += rmap["out"]
    return out


# revision 53
# speedup vs baseline: 1.0086x; 1.0021x over previous
"""Causal group-query attention on 8 Trainium2 NeuronCores (bf16 edition).

Sharding: core c -> (batch b = c // 4, kv-group g = c % 4).
Each core owns batch element b, q-heads [4g, 4g+4) and kv-group g (n_rep = 4,
so those 4 q-heads attend to exactly kv-group g's k/v).  Every core computes
its partial o_proj output (contracting head-concat columns [512g, 512g+512)),
and the host sums the 4 partials per batch element (the "all-reduce after
o_proj" done host-side since we return full outputs anyway).

v2 vs the fp32r baseline (394us):
  * all matmuls in bf16: moving operand streams 2 elem/cycle (vs 1 for
    fp32r) and FWL kicks in for the 128-col weight loads -> ~2x PE.
  * PSUM laid out as 2 wide [128,2,512] tiles + 4 narrow [128,512] tiles
    (8 banks total).  Phase A: qt01/qt23 in the wide tiles, kt/vt narrow.
    Phase B: the wide tiles double-buffer the head-pair score banks, so
    one ACT instruction computes exp for BOTH heads of a pair ([128,2,N]
    3-D AP), halving ACT's 352-cycle per-instruction overhead.
  * both heads' softmax denominators accumulate into ONE narrow bank at
    partitions 0/32 (tile_position=(0,32j)); only the first matmul of the
    pair uses start=True, so the whole-bank has_written clear happens
    exactly once.
  * reciprocal via reciprocal_approx_fast ([1,512], ~5x faster than
    nc.vector.reciprocal) straight from PSUM; the raw->recip->PE
    broadcast->DVE multiply chain never touches ACT.
  * q/k/v bias-adds evacuate PSUM on DVE (tensor_scalar add) instead of
    ACT activations; causal masking runs on the otherwise-idle GpSimd.
  * o_proj of block tb-1 is emitted interleaved with block tb's RoPE so
    the PE always has ready work while ACT/DVE chew the serial chains.
"""

import math

import numpy as np

B, T, D = 2, 2048, 2048
N_HEAD, N_GROUP = 16, 4
HS = D // N_HEAD  # 128
N_REP = N_HEAD // N_GROUP  # 4
NH_C = N_HEAD // N_GROUP  # heads per core = 4
INV_SQRT_HS = 1.0 / math.sqrt(HS)

_NC_CACHE: dict = {}


def build_nc(t=T, dt16="bf16", den_pack=False, recip_psum=True, gp_mask=True,
             pair_oproj=True, den_tile=False):
    """Build and compile the per-core Bass program. Returns the compiled nc."""
    import concourse.bass as bass  # noqa: F401
    import concourse.mybir as mybir
    import concourse.tile as tile
    from concourse import bacc

    f32 = mybir.dt.float32
    f32r = mybir.dt.float32r
    DT = {"bf16": mybir.dt.bfloat16, "fp16": mybir.dt.float16}[dt16]
    exp_f = mybir.ActivationFunctionType.Exp
    ident_f = mybir.ActivationFunctionType.Identity



    nd = D // 128  # d-tiles (contraction) = 16
    tb_n = t // 512  # 512-wide t blocks
    nk = t // 128  # 128-wide k tiles

    nc = bacc.Bacc("TRN2", target_bir_lowering=False, debug=False)

    xd = nc.dram_tensor("x_t", [D, t], DT, kind="ExternalInput")
    wqd = nc.dram_tensor("wq_t", [D, NH_C * HS], DT, kind="ExternalInput")
    wkd = nc.dram_tensor("wk_t", [D, HS], DT, kind="ExternalInput")
    wvd = nc.dram_tensor("wv_t", [D, HS], DT, kind="ExternalInput")
    wod = nc.dram_tensor("wo_t", [NH_C * HS, D], DT, kind="ExternalInput")
    cosd = nc.dram_tensor("cos_t", [HS, t], DT, kind="ExternalInput")
    sind = nc.dram_tensor("sin_t", [HS, t], DT, kind="ExternalInput")
    bqd = nc.dram_tensor("b_q", [HS, NH_C], f32, kind="ExternalInput")
    bkd = nc.dram_tensor("b_k", [HS, 1], f32, kind="ExternalInput")
    bvd = nc.dram_tensor("b_v", [HS, 1], f32, kind="ExternalInput")
    rtd = nc.dram_tensor("r_t", [HS, HS], DT, kind="ExternalInput")
    maskd = nc.dram_tensor("mask_ut", [128, 128], DT, kind="ExternalInput")
    identd = nc.dram_tensor("ident", [128, 128], f32, kind="ExternalInput")
    outd = nc.dram_tensor("out", [t, D], f32, kind="ExternalOutput")

    with tile.TileContext(nc) as tc:
        with (
            tc.tile_pool(name="consts", bufs=1) as consts,
            tc.tile_pool(name="wpool", bufs=1) as wpool,
            tc.tile_pool(name="resid", bufs=1) as resid,
            tc.tile_pool(name="xin", bufs=10) as xin,
            tc.tile_pool(name="work", bufs=4) as work,
            tc.tile_pool(name="vwork", bufs=2) as vwork,
            tc.tile_pool(name="drow", bufs=4) as drow,
            tc.tile_pool(name="qfp", bufs=2) as qfp,
            tc.tile_pool(name="ptp", bufs=4) as ptp,
            tc.tile_pool(name="otp", bufs=8) as otp,
            tc.tile_pool(name="bcp", bufs=2) as bcp,
            tc.tile_pool(name="outp", bufs=4) as outp,
            tc.tile_pool(name="psw", bufs=2, space="PSUM") as psw,
            tc.tile_pool(name="psn", bufs=4, space="PSUM") as psn,
        ):
            def wide(name):
                return psw.tile([128, 2, 512], f32, tag="wide", name=name)

            def nar(name):
                return psn.tile([128, 512], f32, tag="nar", name=name)

            # ---- constants / weights (loaded once) ----
            cos_sb = consts.tile([128, t], DT, name="cos_sb")
            sin_sb = consts.tile([128, t], DT, name="sin_sb")
            rt_sb = consts.tile([128, 128], DT, name="rt_sb")
            mask_sb = consts.tile([128, 128], DT, name="mask_sb")
            id_sb = consts.tile([128, 128], f32, name="id_sb")
            ones16 = consts.tile([128, 1], DT, name="ones16")
            onesf_raw = consts.tile([1, 128], f32, name="onesf_raw")
            onesf = consts.tile([1, 128], f32, name="onesf")
            bq_sb = consts.tile([128, NH_C], f32, name="bq_sb")
            bk_sb = consts.tile([128, 1], f32, name="bk_sb")
            bv_sb = consts.tile([128, 1], f32, name="bv_sb")
            wq_sb = wpool.tile([128, nd, NH_C * HS], DT, name="wq_sb")
            wk_sb = wpool.tile([128, nd, HS], DT, name="wk_sb")
            wv_sb = wpool.tile([128, nd, HS], DT, name="wv_sb")
            wo_sb = wpool.tile([128, NH_C, D], DT, name="wo_sb")
            wq_re = wqd[:, :].rearrange("(n p) m -> p n m", p=128)
            wk_re = wkd[:, :].rearrange("(n p) m -> p n m", p=128)
            wv_re = wvd[:, :].rearrange("(n p) m -> p n m", p=128)

            # resident K^T [hs, t] and V [t(128-tiles), hs]
            kt_sb = resid.tile([128, t], DT, name="kt_sb")
            v_sb = resid.tile([128, nk, HS], DT, name="v_sb")

            x_re = xd[:, :].rearrange("(n p) t -> p n t", p=128)

            oproj_units = []  # deferred (tb, s, ot_sb) work units
            oproj_q = [0]  # alternate output-DMA queue (gpsimd / sync)

            def emit_oproj_unit(evac_eng, split_dma=False):
                # one unit = a full 128-row output block: 4 db sub-blocks of
                # 4 head-accumulated matmuls each, evacuated into one wide
                # [128, 2048] tile and shipped with a single DMA.  With
                # split_dma (tail units) each db half ships separately so the
                # transfer overlaps the remaining evacuations.
                if not oproj_units:
                    return
                tb, s, ot_sb = oproj_units.pop(0)
                ts0 = tb * 512
                ob = outp.tile([128, D], f32, name="ob")
                # db-pairs with h outer: each head's stationary osb slice is
                # loaded once per pair of db sub-blocks instead of per block.
                for dbp in range(2):
                    op_ps = {db: nar("op_ps") for db in (2 * dbp, 2 * dbp + 1)}
                    for h in range(NH_C):
                        for db in (2 * dbp, 2 * dbp + 1):
                            nc.tensor.matmul(
                                op_ps[db],
                                lhsT=ot_sb[h][:, 128 * s: 128 * (s + 1)],
                                rhs=wo_sb[:, h, 512 * db: 512 * (db + 1)],
                                start=h == 0, stop=h == NH_C - 1,
                            )
                    for db in (2 * dbp, 2 * dbp + 1):
                        ob_sl = ob[:, 512 * db: 512 * (db + 1)]
                        if (db % 2 == 0) == (evac_eng == "v"):
                            nc.vector.tensor_copy(out=ob_sl, in_=op_ps[db])
                        else:
                            nc.scalar.copy(out=ob_sl, in_=op_ps[db])
                    if split_dma:
                        eng = nc.gpsimd if oproj_q[0] % 2 == 0 else nc.sync
                        oproj_q[0] += 1
                        eng.dma_start(
                            out=outd[
                                ts0 + 128 * s: ts0 + 128 * (s + 1),
                                1024 * dbp: 1024 * (dbp + 1),
                            ],
                            in_=ob[:, 1024 * dbp: 1024 * (dbp + 1)],
                        )
                if not split_dma:
                    eng = nc.gpsimd if oproj_q[0] % 2 == 0 else nc.sync
                    oproj_q[0] += 1
                    eng.dma_start(
                        out=outd[ts0 + 128 * s: ts0 + 128 * (s + 1), :],
                        in_=ob,
                    )

            def emit_proj(tb):
                # ============ stage 1: q/k/v projection matmuls for block tb
                ts0 = tb * 512
                qt01 = wide("qt01")
                qt23 = wide("qt23")
                kt_ps = nar("kt_ps")
                vt_ps = nar("vt_ps")
                xts_tb = []
                for chunk in range(nd // 2):
                    c2 = 2 * chunk
                    xt = xin.tile([128, 2, 512], DT, name="xt")
                    xts_tb.append(xt)
                    nc.sync.dma_start(
                        out=xt, in_=x_re[:, c2: c2 + 2, ts0: ts0 + 512]
                    )
                    if tb == 0:
                        nc.sync.dma_start(
                            out=wq_sb[:, c2: c2 + 2, :],
                            in_=wq_re[:, c2: c2 + 2, :],
                        )
                        nc.sync.dma_start(
                            out=wk_sb[:, c2: c2 + 2, :],
                            in_=wk_re[:, c2: c2 + 2, :],
                        )
                        nc.sync.dma_start(
                            out=wv_sb[:, c2: c2 + 2, :],
                            in_=wv_re[:, c2: c2 + 2, :],
                        )
                        if chunk == 0:
                            nc.sync.dma_start(out=bq_sb, in_=bqd[:, :])
                            nc.sync.dma_start(out=bk_sb, in_=bkd[:, :])
                            nc.sync.dma_start(out=bv_sb, in_=bvd[:, :])
                            nc.vector.memset(ones16, 1.0)
                            nc.vector.memset(onesf_raw, 1.0)
                            # fp32r matmul operands must come from a
                            # producer with f32r-typed output (walrus rule)
                            nc.vector.tensor_copy(
                                out=onesf[:, :].bitcast(f32r), in_=onesf_raw
                            )
                    for j in range(2):
                        dt = c2 + j
                        first, last = dt == 0, dt == nd - 1
                        for h in range(2):
                            nc.tensor.matmul(
                                qt01[:, h, :],
                                lhsT=wq_sb[:, dt, h * HS: (h + 1) * HS],
                                rhs=xt[:, j, :],
                                start=first, stop=last,
                            )
                        for h in range(2):
                            nc.tensor.matmul(
                                qt23[:, h, :],
                                lhsT=wq_sb[:, dt, (2 + h) * HS: (3 + h) * HS],
                                rhs=xt[:, j, :],
                                start=first, stop=last,
                            )
                        nc.tensor.matmul(
                            kt_ps, lhsT=wk_sb[:, dt, :], rhs=xt[:, j, :],
                            start=first, stop=last,
                        )
                        nc.tensor.matmul(
                            vt_ps, lhsT=wv_sb[:, dt, :], rhs=xt[:, j, :],
                            start=first, stop=last,
                        )

                if tb == 0:
                    # one-time loads go on the gpsimd queue so the sync queue
                    # stays dedicated to the xt stream.  Gate the queue on
                    # chunk 3's arrival so these bulk loads don't steal HBM
                    # bandwidth from the critical first x/w chunks.
                    gate = drow.tile([1, 1], DT, name="gate")
                    nc.gpsimd.tensor_copy(
                        out=gate, in_=xts_tb[3][0:1, 0, 0:1]
                    )
                    nc.gpsimd.dma_start(out=rt_sb, in_=rtd[:, :])
                    nc.gpsimd.dma_start(out=id_sb, in_=identd[:, :])
                    nc.gpsimd.dma_start(out=mask_sb, in_=maskd[:, :])
                    nc.gpsimd.dma_start(out=cos_sb, in_=cosd[:, :])
                    nc.gpsimd.dma_start(out=sin_sb, in_=sind[:, :])
                    wo_re = wod[:, :].rearrange("(h p) m -> p h m", p=128)
                    for h in range(NH_C):
                        nc.gpsimd.dma_start(
                            out=wo_sb[:, h: h + 1, :],
                            in_=wo_re[:, h: h + 1, :],
                        )
                return dict(qt01=qt01, qt23=qt23, kt_ps=kt_ps, vt_ps=vt_ps)

            def emit_evac(tb, stt):
                # ============ stage 2: bias-add PSUM evacuations.  k/v on
                # DVE (the k-rope chain is the critical path), q on ACT so
                # the two chains overlap.  Frees all of stage 1's banks.
                kraw = work.tile([128, 512], DT, name="kraw", bufs=2)
                nc.vector.tensor_scalar_add(
                    out=kraw, in0=stt["kt_ps"], scalar1=bk_sb[:, 0:1]
                )
                vraw = vwork.tile([128, 512], f32, name="vraw")
                nc.vector.tensor_scalar_add(
                    out=vraw, in0=stt["vt_ps"], scalar1=bv_sb[:, 0:1]
                )
                qraws = []
                for h in range(NH_C):
                    qraw = work.tile([128, 512], DT, name="qraw")
                    src = (stt["qt01"][:, h, :] if h < 2
                           else stt["qt23"][:, h - 2, :])
                    nc.scalar.activation(
                        out=qraw, in_=src, func=ident_f,
                        bias=bq_sb[:, h: h + 1], scale=1.0,
                    )
                    qraws.append(qraw)
                stt.update(kraw=kraw, vraw=vraw, qraws=qraws)

            def emit_rope(tb, stt):
                # ============ stage 3: RoPE (k then q) + V transpose.
                # o_proj units of the previous block keep the PE fed while
                # ACT/DVE chew the serial chains.
                ts0 = tb * 512
                kraw, vraw, qraws = stt["kraw"], stt["vraw"], stt["qraws"]
                qf = qfp.tile([128, NH_C, 512], DT, name="qf")
                rot_k = nar("rot_ps")
                nc.tensor.matmul(
                    rot_k, lhsT=rt_sb[:, :], rhs=kraw, start=True, stop=True
                )
                if not pair_oproj:
                    emit_oproj_unit("s")
                nc.vector.tensor_mul(
                    kt_sb[:, ts0: ts0 + 512], kraw, cos_sb[:, ts0: ts0 + 512]
                )
                rtmp = work.tile([128, 512], DT, name="rtmp", bufs=2)
                nc.vector.tensor_mul(rtmp, rot_k, sin_sb[:, ts0: ts0 + 512])
                nc.vector.tensor_add(
                    kt_sb[:, ts0: ts0 + 512], kt_sb[:, ts0: ts0 + 512], rtmp
                )

                # v: transpose [hs, t] -> [t, hs] tiles
                for s in range(4):
                    vt_tp = nar("vt_tp")
                    nc.tensor.transpose(
                        vt_tp[:, 0:128], vraw[:, 128 * s: 128 * (s + 1)],
                        id_sb[:, :],
                    )
                    nc.vector.tensor_copy(
                        out=v_sb[:, 4 * tb + s, :], in_=vt_tp[:, 0:128]
                    )

                rot_ps = {}
                for h in range(NH_C):
                    rot_ps[h] = nar("rot_ps")
                    nc.tensor.matmul(
                        rot_ps[h], lhsT=rt_sb[:, :], rhs=qraws[h],
                        start=True, stop=True,
                    )
                    if h % 2 == 1:
                        emit_oproj_unit("v" if h == 1 else "s")
                    nc.vector.tensor_mul(
                        qf[:, h, :], qraws[h], cos_sb[:, ts0: ts0 + 512]
                    )
                    rtmp = work.tile([128, 512], DT, name="rtmp", bufs=2)
                    nc.vector.tensor_mul(
                        rtmp, rot_ps[h], sin_sb[:, ts0: ts0 + 512]
                    )
                    nc.vector.tensor_add(qf[:, h, :], qf[:, h, :], rtmp)
                if not pair_oproj:
                    emit_oproj_unit("v")
                stt["qf"] = qf

            def emit_attn(tb, stt):
                # ============ stage 4: attention for q-block jq == tb
                # Heads in pairs; both heads' scores live in one wide PSUM
                # tile so exp is a single [128, 2, N] ACT op.  PV/denominator
                # of k-tile i-1 issue while ACT computes exp of tile i.
                qf = stt["qf"]
                ot_sb = {}
                imax = 4 * tb + 3
                for hp in range(NH_C // 2):
                    heads = (2 * hp, 2 * hp + 1)
                    ot_ps = {h: nar(f"ot_ps{h}") for h in heads}
                    if den_pack:
                        den_ps = nar("den_ps")
                        den_ap = {heads[0]: den_ps[0:1, :],
                                  heads[1]: den_ps[32:33, :]}
                        den_tp = {heads[0]: None, heads[1]: (0, 32)}
                        den_start = {heads[0]: lambda i: i == 0,
                                     heads[1]: lambda i: False}
                    else:
                        # separate banks per head, but head 1's row sits at
                        # partition 32 via tile_position so the two den
                        # matmuls target different PE column groups (they can
                        # overlap in the array).
                        dps = {h: nar(f"den_ps{h}") for h in heads}
                        if den_tile:
                            den_ap = {heads[0]: dps[heads[0]][0:1, :],
                                      heads[1]: dps[heads[1]][32:33, :]}
                            den_tp = {heads[0]: None, heads[1]: (0, 32)}
                        else:
                            den_ap = {h: dps[h][0:1, :] for h in heads}
                            den_tp = {h: None for h in heads}
                        den_start = {h: (lambda i: i == 0) for h in heads}

                    def emit_pv_den(i, pt, c0):
                        # dens first (reciprocal chain starts earlier) and
                        # adjacent (they share the ones16 stationary operand),
                        # then both pvs (they share the v tile).
                        first, last = i == 0, i == imax
                        for j, h in enumerate(heads):
                            kw = {}
                            if den_tp[h] is not None:
                                kw["tile_position"] = den_tp[h]
                            nc.tensor.matmul(
                                den_ap[h][:, c0:],
                                lhsT=ones16[:, 0:1],
                                rhs=pt[:, j, c0:],
                                start=den_start[h](i), stop=last,
                                skip_group_check=True,
                                **kw,
                            )
                        for j, h in enumerate(heads):
                            nc.tensor.matmul(
                                ot_ps[h][:, c0:],
                                lhsT=v_sb[:, i, :],
                                rhs=pt[:, j, c0:],
                                start=first, stop=last,
                            )

                    prev = None
                    for i in range(imax + 1):
                        c0 = 128 * max(0, i - 4 * tb)
                        diag = i >= 4 * tb
                        st = wide("st")
                        for j, h in enumerate(heads):
                            nc.tensor.matmul(
                                st[:, j, c0:],
                                lhsT=kt_sb[:, 128 * i: 128 * (i + 1)],
                                rhs=qf[:, h, c0:],
                                start=True, stop=True,
                            )
                        pt = ptp.tile([128, 2, 512], DT, name="pt")
                        nc.scalar.activation(
                            out=pt[:, :, c0:], in_=st[:, :, c0:], func=exp_f,
                            scale=INV_SQRT_HS,
                        )
                        if diag:
                            for j in range(2):
                                if gp_mask:
                                    # zero strictly-lower triangle (tk > tq)
                                    nc.gpsimd.affine_select(
                                        out=pt[:, j, c0: c0 + 128],
                                        in_=pt[:, j, c0: c0 + 128],
                                        compare_op=mybir.AluOpType.is_ge,
                                        fill=0.0,
                                        base=0,
                                        pattern=[[1, 128]],
                                        channel_multiplier=-1,
                                    )
                                else:
                                    nc.vector.tensor_mul(
                                        pt[:, j, c0: c0 + 128],
                                        pt[:, j, c0: c0 + 128],
                                        mask_sb,
                                    )
                        if prev is not None:
                            emit_pv_den(*prev)
                        prev = (i, pt, c0)
                    emit_pv_den(*prev)

                    # normalize each head's O^T by its softmax denominator:
                    # 1/den straight from PSUM (reciprocal_approx_fast) frees
                    # the den banks at once; ACT evacuates O^T (freeing the ot
                    # banks for the next pair); PE broadcasts the reciprocal
                    # across partitions and DVE multiplies it in-place,
                    # reading the broadcast directly from PSUM.
                    denrec = {}
                    for j, h in enumerate(heads):
                        draw = drow.tile([1, 512], f32, name="draw")
                        if recip_psum:
                            nc.vector.reciprocal_approx_fast(
                                out=draw, in_=den_ap[h]
                            )
                        else:
                            dcp = drow.tile([1, 512], f32, name="dcp")
                            nc.scalar.copy(out=dcp, in_=den_ap[h])
                            nc.vector.reciprocal_approx_fast(
                                out=draw, in_=dcp
                            )
                        denrec[h] = drow.tile([1, 512], f32, name="denrec")
                        nc.vector.tensor_copy(
                            out=denrec[h][:, :].bitcast(f32r), in_=draw
                        )
                    for j, h in enumerate(heads):
                        osb = otp.tile([128, 512], DT, name="osb")
                        nc.scalar.copy(out=osb, in_=ot_ps[h])
                        ot_sb[h] = osb
                    for j, h in enumerate(heads):
                        bc_ps = nar("bc_ps")
                        nc.tensor.matmul(
                            bc_ps,
                            lhsT=onesf[:, :].bitcast(f32r),
                            rhs=denrec[h][:, :].bitcast(f32r),
                            start=True, stop=True,
                        )
                        nc.vector.tensor_mul(ot_sb[h], ot_sb[h], bc_ps)
                    if pair_oproj:
                        # all pair banks are released here: an o_proj unit
                        # fills the PE while the next pair's chains warm up
                        emit_oproj_unit("s" if hp == 0 else "v")

                for s in range(4):
                    oproj_units.append((tb, s, ot_sb))

            # ============ pipelined schedule: projections of block tb+1 are
            # emitted before attention of block tb, so the PE's projection
            # stream covers the serial evac/rope chains on ACT/DVE, and the
            # rope of tb+1 lands after attention of tb where the previous
            # block's o_proj units fill the rot-matmul waits.
            # warm-up: dependency-free matmuls keep the PE busy while the
            # first DMA chunks land, so the HAM clock-gate is already at
            # 8/8 when the real matmuls start (cold matmuls run at 1.2GHz).
            warm_sb = consts.tile([128, 64], DT, name="warm_sb")
            nc.vector.memset(warm_sb, 1.0)
            warm_ps = nar("warm_ps")
            for _ in range(20):
                nc.tensor.matmul(
                    warm_ps[0:64, 0:64], lhsT=warm_sb[:, 0:64],
                    rhs=warm_sb[:, 0:64], start=True, stop=True,
                    skip_group_check=True,
                )

            states = {}
            states[0] = emit_proj(0)
            emit_evac(0, states[0])
            emit_rope(0, states[0])
            for tb in range(tb_n):
                if tb + 1 < tb_n:
                    states[tb + 1] = emit_proj(tb + 1)
                    emit_evac(tb + 1, states[tb + 1])
                emit_attn(tb, states[tb])
                if tb + 1 < tb_n:
                    emit_rope(tb + 1, states[tb + 1])
                    states.pop(tb)

            while oproj_units:
                emit_oproj_unit("s", split_dma=True)
                emit_oproj_unit("v", split_dma=True)

    nc.compile()
    return nc


def shard_inputs(x, cos, sin, Wq, bq, Wkv, bkv, Wo, t=T, dt16="bf16"):
    """Build the 8 per-core input maps (core c -> batch c//4, group c%4)."""
    import ml_dtypes

    DT = {"bf16": ml_dtypes.bfloat16, "fp16": np.float16}[dt16]
    f32 = np.float32
    hs = HS
    rot = np.zeros((hs, hs), f32)
    for i in range(hs // 2):
        rot[i, i + hs // 2] = -1.0
        rot[i + hs // 2, i] = 1.0
    r_t = np.ascontiguousarray(rot.T).astype(DT)
    mask_ut = np.triu(np.ones((128, 128), f32)).astype(DT)
    ident = np.eye(128, dtype=f32)
    cos_t = np.ascontiguousarray(cos.T).astype(DT)
    sin_t = np.ascontiguousarray(sin.T).astype(DT)

    xts = [np.ascontiguousarray(x[b].T).astype(DT) for b in range(x.shape[0])]
    per_g = []
    for g in range(4):
        per_g.append(
            dict(
                wq_t=np.ascontiguousarray(
                    Wq[512 * g: 512 * g + 512].T).astype(DT),
                b_q=np.ascontiguousarray(
                    bq[512 * g: 512 * g + 512].reshape(4, 128).T.astype(f32)
                ),
                wk_t=np.ascontiguousarray(
                    Wkv[128 * g: 128 * g + 128].T).astype(DT),
                b_k=np.ascontiguousarray(
                    bkv[128 * g: 128 * g + 128].reshape(128, 1).astype(f32)
                ),
                wv_t=np.ascontiguousarray(
                    Wkv[512 + 128 * g: 512 + 128 * g + 128].T).astype(DT),
                b_v=np.ascontiguousarray(
                    bkv[512 + 128 * g: 512 + 128 * g + 128]
                    .reshape(128, 1)
                    .astype(f32)
                ),
                wo_t=np.ascontiguousarray(
                    Wo[:, 512 * g: 512 * g + 512].T).astype(DT),
            )
        )

    in_maps = []
    for c in range(4 * x.shape[0]):
        b, g = c // 4, c % 4
        m = dict(per_g[g])
        m.update(
            x_t=xts[b], cos_t=cos_t, sin_t=sin_t,
            r_t=r_t, mask_ut=mask_ut, ident=ident,
        )
        in_maps.append(m)
    return in_maps


def run_on_hw(in_maps, t=T, trace=False, **flags):
    from concourse.bass_utils import run_bass_kernel_spmd

    key = (t, tuple(sorted(flags.items())))
    if key not in _NC_CACHE:
        _NC_CACHE[key] = build_nc(t, **flags)
    nc = _NC_CACHE[key]
    res = run_bass_kernel_spmd(
        nc, in_maps, core_ids=list(range(len(in_maps))), trace=trace
    )
    return res


def kernel(x, cos, sin, Wq, bq, Wkv, bkv, Wo):
    x = np.asarray(x)
    in_maps = shard_inputs(
        x, np.asarray(cos), np.asarray(sin), np.asarray(Wq), np.asarray(bq),
        np.asarray(Wkv), np.asarray(bkv), np.asarray(Wo),
    )
    res = run_on_hw(in_maps, t=T, trace=False)
    out = np.zeros((B, T, D), np.float32)
    for c, rmap in enumerate(res.results):
        out[c // 4] += rmap["out"]
    return out


# revision 54
# speedup vs baseline: 1.0092x; 1.0006x over previous
"""Causal group-query attention on 8 Trainium2 NeuronCores (bf16 edition).

Sharding: core c -> (batch b = c // 4, kv-group g = c % 4).
Each core owns batch element b, q-heads [4g, 4g+4) and kv-group g (n_rep = 4,
so those 4 q-heads attend to exactly kv-group g's k/v).  Every core computes
its partial o_proj output (contracting head-concat columns [512g, 512g+512)),
and the host sums the 4 partials per batch element (the "all-reduce after
o_proj" done host-side since we return full outputs anyway).

v2 vs the fp32r baseline (394us):
  * all matmuls in bf16: moving operand streams 2 elem/cycle (vs 1 for
    fp32r) and FWL kicks in for the 128-col weight loads -> ~2x PE.
  * PSUM laid out as 2 wide [128,2,512] tiles + 4 narrow [128,512] tiles
    (8 banks total).  Phase A: qt01/qt23 in the wide tiles, kt/vt narrow.
    Phase B: the wide tiles double-buffer the head-pair score banks, so
    one ACT instruction computes exp for BOTH heads of a pair ([128,2,N]
    3-D AP), halving ACT's 352-cycle per-instruction overhead.
  * both heads' softmax denominators accumulate into ONE narrow bank at
    partitions 0/32 (tile_position=(0,32j)); only the first matmul of the
    pair uses start=True, so the whole-bank has_written clear happens
    exactly once.
  * reciprocal via reciprocal_approx_fast ([1,512], ~5x faster than
    nc.vector.reciprocal) straight from PSUM; the raw->recip->PE
    broadcast->DVE multiply chain never touches ACT.
  * q/k/v bias-adds evacuate PSUM on DVE (tensor_scalar add) instead of
    ACT activations; causal masking runs on the otherwise-idle GpSimd.
  * o_proj of block tb-1 is emitted interleaved with block tb's RoPE so
    the PE always has ready work while ACT/DVE chew the serial chains.
"""

import math

import numpy as np

B, T, D = 2, 2048, 2048
N_HEAD, N_GROUP = 16, 4
HS = D // N_HEAD  # 128
N_REP = N_HEAD // N_GROUP  # 4
NH_C = N_HEAD // N_GROUP  # heads per core = 4
INV_SQRT_HS = 1.0 / math.sqrt(HS)

_NC_CACHE: dict = {}


def build_nc(t=T, dt16="bf16", den_pack=False, recip_psum=True, gp_mask=True,
             pair_oproj=True, den_tile=False):
    """Build and compile the per-core Bass program. Returns the compiled nc."""
    import concourse.bass as bass  # noqa: F401
    import concourse.mybir as mybir
    import concourse.tile as tile
    from concourse import bacc

    f32 = mybir.dt.float32
    f32r = mybir.dt.float32r
    DT = {"bf16": mybir.dt.bfloat16, "fp16": mybir.dt.float16}[dt16]
    exp_f = mybir.ActivationFunctionType.Exp
    ident_f = mybir.ActivationFunctionType.Identity



    nd = D // 128  # d-tiles (contraction) = 16
    tb_n = t // 512  # 512-wide t blocks
    nk = t // 128  # 128-wide k tiles

    nc = bacc.Bacc("TRN2", target_bir_lowering=False, debug=False)

    xd = nc.dram_tensor("x_t", [D, t], DT, kind="ExternalInput")
    wqd = nc.dram_tensor("wq_t", [D, NH_C * HS], DT, kind="ExternalInput")
    wkd = nc.dram_tensor("wk_t", [D, HS], DT, kind="ExternalInput")
    wvd = nc.dram_tensor("wv_t", [D, HS], DT, kind="ExternalInput")
    wod = nc.dram_tensor("wo_t", [NH_C * HS, D], DT, kind="ExternalInput")
    cosd = nc.dram_tensor("cos_t", [HS, t], DT, kind="ExternalInput")
    sind = nc.dram_tensor("sin_t", [HS, t], DT, kind="ExternalInput")
    bqd = nc.dram_tensor("b_q", [HS, NH_C], f32, kind="ExternalInput")
    bkd = nc.dram_tensor("b_k", [HS, 1], f32, kind="ExternalInput")
    bvd = nc.dram_tensor("b_v", [HS, 1], f32, kind="ExternalInput")
    rtd = nc.dram_tensor("r_t", [HS, HS], DT, kind="ExternalInput")
    maskd = nc.dram_tensor("mask_ut", [128, 128], DT, kind="ExternalInput")
    identd = nc.dram_tensor("ident", [128, 128], f32, kind="ExternalInput")
    outd = nc.dram_tensor("out", [t, D], f32, kind="ExternalOutput")

    with tile.TileContext(nc) as tc:
        with (
            tc.tile_pool(name="consts", bufs=1) as consts,
            tc.tile_pool(name="wpool", bufs=1) as wpool,
            tc.tile_pool(name="resid", bufs=1) as resid,
            tc.tile_pool(name="xin", bufs=10) as xin,
            tc.tile_pool(name="work", bufs=4) as work,
            tc.tile_pool(name="vwork", bufs=2) as vwork,
            tc.tile_pool(name="drow", bufs=4) as drow,
            tc.tile_pool(name="qfp", bufs=2) as qfp,
            tc.tile_pool(name="ptp", bufs=6) as ptp,
            tc.tile_pool(name="otp", bufs=8) as otp,
            tc.tile_pool(name="bcp", bufs=2) as bcp,
            tc.tile_pool(name="outp", bufs=4) as outp,
            tc.tile_pool(name="psw", bufs=2, space="PSUM") as psw,
            tc.tile_pool(name="psn", bufs=4, space="PSUM") as psn,
        ):
            def wide(name):
                return psw.tile([128, 2, 512], f32, tag="wide", name=name)

            def nar(name):
                return psn.tile([128, 512], f32, tag="nar", name=name)

            # ---- constants / weights (loaded once) ----
            cos_sb = consts.tile([128, t], DT, name="cos_sb")
            sin_sb = consts.tile([128, t], DT, name="sin_sb")
            rt_sb = consts.tile([128, 128], DT, name="rt_sb")
            mask_sb = consts.tile([128, 128], DT, name="mask_sb")
            id_sb = consts.tile([128, 128], f32, name="id_sb")
            ones16 = consts.tile([128, 1], DT, name="ones16")
            onesf_raw = consts.tile([1, 128], f32, name="onesf_raw")
            onesf = consts.tile([1, 128], f32, name="onesf")
            bq_sb = consts.tile([128, NH_C], f32, name="bq_sb")
            bk_sb = consts.tile([128, 1], f32, name="bk_sb")
            bv_sb = consts.tile([128, 1], f32, name="bv_sb")
            wq_sb = wpool.tile([128, nd, NH_C * HS], DT, name="wq_sb")
            wk_sb = wpool.tile([128, nd, HS], DT, name="wk_sb")
            wv_sb = wpool.tile([128, nd, HS], DT, name="wv_sb")
            wo_sb = wpool.tile([128, NH_C, D], DT, name="wo_sb")
            wq_re = wqd[:, :].rearrange("(n p) m -> p n m", p=128)
            wk_re = wkd[:, :].rearrange("(n p) m -> p n m", p=128)
            wv_re = wvd[:, :].rearrange("(n p) m -> p n m", p=128)

            # resident K^T [hs, t] and V [t(128-tiles), hs]
            kt_sb = resid.tile([128, t], DT, name="kt_sb")
            v_sb = resid.tile([128, nk, HS], DT, name="v_sb")

            x_re = xd[:, :].rearrange("(n p) t -> p n t", p=128)

            oproj_units = []  # deferred (tb, s, ot_sb) work units
            oproj_q = [0]  # alternate output-DMA queue (gpsimd / sync)

            def emit_oproj_unit(evac_eng, split_dma=False):
                # one unit = a full 128-row output block: 4 db sub-blocks of
                # 4 head-accumulated matmuls each, evacuated into one wide
                # [128, 2048] tile and shipped with a single DMA.  With
                # split_dma (tail units) each db half ships separately so the
                # transfer overlaps the remaining evacuations.
                if not oproj_units:
                    return
                tb, s, ot_sb = oproj_units.pop(0)
                ts0 = tb * 512
                ob = outp.tile([128, D], f32, name="ob")
                # db-pairs with h outer: each head's stationary osb slice is
                # loaded once per pair of db sub-blocks instead of per block.
                for dbp in range(2):
                    op_ps = {db: nar("op_ps") for db in (2 * dbp, 2 * dbp + 1)}
                    for h in range(NH_C):
                        for db in (2 * dbp, 2 * dbp + 1):
                            nc.tensor.matmul(
                                op_ps[db],
                                lhsT=ot_sb[h][:, 128 * s: 128 * (s + 1)],
                                rhs=wo_sb[:, h, 512 * db: 512 * (db + 1)],
                                start=h == 0, stop=h == NH_C - 1,
                            )
                    for db in (2 * dbp, 2 * dbp + 1):
                        ob_sl = ob[:, 512 * db: 512 * (db + 1)]
                        if (db % 2 == 0) == (evac_eng == "v"):
                            nc.vector.tensor_copy(out=ob_sl, in_=op_ps[db])
                        else:
                            nc.scalar.copy(out=ob_sl, in_=op_ps[db])
                    if split_dma:
                        eng = nc.gpsimd if oproj_q[0] % 2 == 0 else nc.sync
                        oproj_q[0] += 1
                        eng.dma_start(
                            out=outd[
                                ts0 + 128 * s: ts0 + 128 * (s + 1),
                                1024 * dbp: 1024 * (dbp + 1),
                            ],
                            in_=ob[:, 1024 * dbp: 1024 * (dbp + 1)],
                        )
                if not split_dma:
                    eng = nc.gpsimd if oproj_q[0] % 2 == 0 else nc.sync
                    oproj_q[0] += 1
                    eng.dma_start(
                        out=outd[ts0 + 128 * s: ts0 + 128 * (s + 1), :],
                        in_=ob,
                    )

            def emit_proj(tb):
                # ============ stage 1: q/k/v projection matmuls for block tb
                ts0 = tb * 512
                qt01 = wide("qt01")
                qt23 = wide("qt23")
                kt_ps = nar("kt_ps")
                vt_ps = nar("vt_ps")
                xts_tb = []
                for chunk in range(nd // 2):
                    c2 = 2 * chunk
                    xt = xin.tile([128, 2, 512], DT, name="xt")
                    xts_tb.append(xt)
                    nc.sync.dma_start(
                        out=xt, in_=x_re[:, c2: c2 + 2, ts0: ts0 + 512]
                    )
                    if tb == 0:
                        nc.sync.dma_start(
                            out=wq_sb[:, c2: c2 + 2, :],
                            in_=wq_re[:, c2: c2 + 2, :],
                        )
                        nc.sync.dma_start(
                            out=wk_sb[:, c2: c2 + 2, :],
                            in_=wk_re[:, c2: c2 + 2, :],
                        )
                        nc.sync.dma_start(
                            out=wv_sb[:, c2: c2 + 2, :],
                            in_=wv_re[:, c2: c2 + 2, :],
                        )
                        if chunk == 0:
                            nc.sync.dma_start(out=bq_sb, in_=bqd[:, :])
                            nc.sync.dma_start(out=bk_sb, in_=bkd[:, :])
                            nc.sync.dma_start(out=bv_sb, in_=bvd[:, :])
                            nc.vector.memset(ones16, 1.0)
                            nc.vector.memset(onesf_raw, 1.0)
                            # fp32r matmul operands must come from a
                            # producer with f32r-typed output (walrus rule)
                            nc.vector.tensor_copy(
                                out=onesf[:, :].bitcast(f32r), in_=onesf_raw
                            )
                    for j in range(2):
                        dt = c2 + j
                        first, last = dt == 0, dt == nd - 1
                        for h in range(2):
                            nc.tensor.matmul(
                                qt01[:, h, :],
                                lhsT=wq_sb[:, dt, h * HS: (h + 1) * HS],
                                rhs=xt[:, j, :],
                                start=first, stop=last,
                            )
                        for h in range(2):
                            nc.tensor.matmul(
                                qt23[:, h, :],
                                lhsT=wq_sb[:, dt, (2 + h) * HS: (3 + h) * HS],
                                rhs=xt[:, j, :],
                                start=first, stop=last,
                            )
                        nc.tensor.matmul(
                            kt_ps, lhsT=wk_sb[:, dt, :], rhs=xt[:, j, :],
                            start=first, stop=last,
                        )
                        nc.tensor.matmul(
                            vt_ps, lhsT=wv_sb[:, dt, :], rhs=xt[:, j, :],
                            start=first, stop=last,
                        )

                if tb == 0:
                    # one-time loads go on the gpsimd queue so the sync queue
                    # stays dedicated to the xt stream.  Gate the queue on
                    # chunk 3's arrival so these bulk loads don't steal HBM
                    # bandwidth from the critical first x/w chunks.
                    gate = drow.tile([1, 1], DT, name="gate")
                    nc.gpsimd.tensor_copy(
                        out=gate, in_=xts_tb[3][0:1, 0, 0:1]
                    )
                    nc.gpsimd.dma_start(out=rt_sb, in_=rtd[:, :])
                    nc.gpsimd.dma_start(out=id_sb, in_=identd[:, :])
                    nc.gpsimd.dma_start(out=mask_sb, in_=maskd[:, :])
                    nc.gpsimd.dma_start(out=cos_sb, in_=cosd[:, :])
                    nc.gpsimd.dma_start(out=sin_sb, in_=sind[:, :])
                    wo_re = wod[:, :].rearrange("(h p) m -> p h m", p=128)
                    for h in range(NH_C):
                        nc.gpsimd.dma_start(
                            out=wo_sb[:, h: h + 1, :],
                            in_=wo_re[:, h: h + 1, :],
                        )
                return dict(qt01=qt01, qt23=qt23, kt_ps=kt_ps, vt_ps=vt_ps)

            def emit_evac(tb, stt):
                # ============ stage 2: bias-add PSUM evacuations.  k/v on
                # DVE (the k-rope chain is the critical path), q on ACT so
                # the two chains overlap.  Frees all of stage 1's banks.
                kraw = work.tile([128, 512], DT, name="kraw", bufs=2)
                nc.vector.tensor_scalar_add(
                    out=kraw, in0=stt["kt_ps"], scalar1=bk_sb[:, 0:1]
                )
                vraw = vwork.tile([128, 512], f32, name="vraw")
                nc.vector.tensor_scalar_add(
                    out=vraw, in0=stt["vt_ps"], scalar1=bv_sb[:, 0:1]
                )
                qraws = []
                for h in range(NH_C):
                    qraw = work.tile([128, 512], DT, name="qraw")
                    src = (stt["qt01"][:, h, :] if h < 2
                           else stt["qt23"][:, h - 2, :])
                    nc.scalar.activation(
                        out=qraw, in_=src, func=ident_f,
                        bias=bq_sb[:, h: h + 1], scale=1.0,
                    )
                    qraws.append(qraw)
                stt.update(kraw=kraw, vraw=vraw, qraws=qraws)

            def emit_rope(tb, stt):
                # ============ stage 3: RoPE (k then q) + V transpose.
                # o_proj units of the previous block keep the PE fed while
                # ACT/DVE chew the serial chains.
                ts0 = tb * 512
                kraw, vraw, qraws = stt["kraw"], stt["vraw"], stt["qraws"]
                qf = qfp.tile([128, NH_C, 512], DT, name="qf")
                rot_k = nar("rot_ps")
                nc.tensor.matmul(
                    rot_k, lhsT=rt_sb[:, :], rhs=kraw, start=True, stop=True
                )
                if not pair_oproj:
                    emit_oproj_unit("s")
                nc.vector.tensor_mul(
                    kt_sb[:, ts0: ts0 + 512], kraw, cos_sb[:, ts0: ts0 + 512]
                )
                rtmp = work.tile([128, 512], DT, name="rtmp", bufs=2)
                nc.vector.tensor_mul(rtmp, rot_k, sin_sb[:, ts0: ts0 + 512])
                nc.vector.tensor_add(
                    kt_sb[:, ts0: ts0 + 512], kt_sb[:, ts0: ts0 + 512], rtmp
                )

                # v: transpose [hs, t] -> [t, hs] tiles
                for s in range(4):
                    vt_tp = nar("vt_tp")
                    nc.tensor.transpose(
                        vt_tp[:, 0:128], vraw[:, 128 * s: 128 * (s + 1)],
                        id_sb[:, :],
                    )
                    nc.vector.tensor_copy(
                        out=v_sb[:, 4 * tb + s, :], in_=vt_tp[:, 0:128]
                    )

                rot_ps = {}
                for h in range(NH_C):
                    rot_ps[h] = nar("rot_ps")
                    nc.tensor.matmul(
                        rot_ps[h], lhsT=rt_sb[:, :], rhs=qraws[h],
                        start=True, stop=True,
                    )
                    if h % 2 == 1:
                        emit_oproj_unit("v" if h == 1 else "s")
                    nc.vector.tensor_mul(
                        qf[:, h, :], qraws[h], cos_sb[:, ts0: ts0 + 512]
                    )
                    rtmp = work.tile([128, 512], DT, name="rtmp", bufs=2)
                    nc.vector.tensor_mul(
                        rtmp, rot_ps[h], sin_sb[:, ts0: ts0 + 512]
                    )
                    nc.vector.tensor_add(qf[:, h, :], qf[:, h, :], rtmp)
                if not pair_oproj:
                    emit_oproj_unit("v")
                stt["qf"] = qf

            def emit_attn(tb, stt):
                # ============ stage 4: attention for q-block jq == tb
                # Heads in pairs; both heads' scores live in one wide PSUM
                # tile so exp is a single [128, 2, N] ACT op.  PV/denominator
                # of k-tile i-1 issue while ACT computes exp of tile i.
                qf = stt["qf"]
                ot_sb = {}
                imax = 4 * tb + 3
                for hp in range(NH_C // 2):
                    heads = (2 * hp, 2 * hp + 1)
                    ot_ps = {h: nar(f"ot_ps{h}") for h in heads}
                    if den_pack:
                        den_ps = nar("den_ps")
                        den_ap = {heads[0]: den_ps[0:1, :],
                                  heads[1]: den_ps[32:33, :]}
                        den_tp = {heads[0]: None, heads[1]: (0, 32)}
                        den_start = {heads[0]: lambda i: i == 0,
                                     heads[1]: lambda i: False}
                    else:
                        # separate banks per head, but head 1's row sits at
                        # partition 32 via tile_position so the two den
                        # matmuls target different PE column groups (they can
                        # overlap in the array).
                        dps = {h: nar(f"den_ps{h}") for h in heads}
                        if den_tile:
                            den_ap = {heads[0]: dps[heads[0]][0:1, :],
                                      heads[1]: dps[heads[1]][32:33, :]}
                            den_tp = {heads[0]: None, heads[1]: (0, 32)}
                        else:
                            den_ap = {h: dps[h][0:1, :] for h in heads}
                            den_tp = {h: None for h in heads}
                        den_start = {h: (lambda i: i == 0) for h in heads}

                    def emit_pv_den(i, pt, c0):
                        # dens first (reciprocal chain starts earlier) and
                        # adjacent (they share the ones16 stationary operand),
                        # then both pvs (they share the v tile).
                        first, last = i == 0, i == imax
                        for j, h in enumerate(heads):
                            kw = {}
                            if den_tp[h] is not None:
                                kw["tile_position"] = den_tp[h]
                            nc.tensor.matmul(
                                den_ap[h][:, c0:],
                                lhsT=ones16[:, 0:1],
                                rhs=pt[:, j, c0:],
                                start=den_start[h](i), stop=last,
                                skip_group_check=True,
                                **kw,
                            )
                        for j, h in enumerate(heads):
                            nc.tensor.matmul(
                                ot_ps[h][:, c0:],
                                lhsT=v_sb[:, i, :],
                                rhs=pt[:, j, c0:],
                                start=first, stop=last,
                            )

                    prev = None
                    for i in range(imax + 1):
                        c0 = 128 * max(0, i - 4 * tb)
                        diag = i >= 4 * tb
                        st = wide("st")
                        for j, h in enumerate(heads):
                            nc.tensor.matmul(
                                st[:, j, c0:],
                                lhsT=kt_sb[:, 128 * i: 128 * (i + 1)],
                                rhs=qf[:, h, c0:],
                                start=True, stop=True,
                            )
                        pt = ptp.tile([128, 2, 512], DT, name="pt")
                        nc.scalar.activation(
                            out=pt[:, :, c0:], in_=st[:, :, c0:], func=exp_f,
                            scale=INV_SQRT_HS,
                        )
                        if diag:
                            for j in range(2):
                                if gp_mask:
                                    # zero strictly-lower triangle (tk > tq)
                                    nc.gpsimd.affine_select(
                                        out=pt[:, j, c0: c0 + 128],
                                        in_=pt[:, j, c0: c0 + 128],
                                        compare_op=mybir.AluOpType.is_ge,
                                        fill=0.0,
                                        base=0,
                                        pattern=[[1, 128]],
                                        channel_multiplier=-1,
                                    )
                                else:
                                    nc.vector.tensor_mul(
                                        pt[:, j, c0: c0 + 128],
                                        pt[:, j, c0: c0 + 128],
                                        mask_sb,
                                    )
                        if prev is not None:
                            emit_pv_den(*prev)
                        prev = (i, pt, c0)
                    emit_pv_den(*prev)

                    # normalize each head's O^T by its softmax denominator:
                    # 1/den straight from PSUM (reciprocal_approx_fast) frees
                    # the den banks at once; ACT evacuates O^T (freeing the ot
                    # banks for the next pair); PE broadcasts the reciprocal
                    # across partitions and DVE multiplies it in-place,
                    # reading the broadcast directly from PSUM.
                    denrec = {}
                    for j, h in enumerate(heads):
                        draw = drow.tile([1, 512], f32, name="draw")
                        if recip_psum:
                            nc.vector.reciprocal_approx_fast(
                                out=draw, in_=den_ap[h]
                            )
                        else:
                            dcp = drow.tile([1, 512], f32, name="dcp")
                            nc.scalar.copy(out=dcp, in_=den_ap[h])
                            nc.vector.reciprocal_approx_fast(
                                out=draw, in_=dcp
                            )
                        denrec[h] = drow.tile([1, 512], f32, name="denrec")
                        nc.vector.tensor_copy(
                            out=denrec[h][:, :].bitcast(f32r), in_=draw
                        )
                    for j, h in enumerate(heads):
                        osb = otp.tile([128, 512], DT, name="osb")
                        nc.scalar.copy(out=osb, in_=ot_ps[h])
                        ot_sb[h] = osb
                    for j, h in enumerate(heads):
                        bc_ps = nar("bc_ps")
                        nc.tensor.matmul(
                            bc_ps,
                            lhsT=onesf[:, :].bitcast(f32r),
                            rhs=denrec[h][:, :].bitcast(f32r),
                            start=True, stop=True,
                        )
                        nc.vector.tensor_mul(ot_sb[h], ot_sb[h], bc_ps)
                    if pair_oproj:
                        # all pair banks are released here: an o_proj unit
                        # fills the PE while the next pair's chains warm up
                        emit_oproj_unit("s" if hp == 0 else "v")

                for s in range(4):
                    oproj_units.append((tb, s, ot_sb))

            # ============ pipelined schedule: projections of block tb+1 are
            # emitted before attention of block tb, so the PE's projection
            # stream covers the serial evac/rope chains on ACT/DVE, and the
            # rope of tb+1 lands after attention of tb where the previous
            # block's o_proj units fill the rot-matmul waits.
            # warm-up: dependency-free matmuls keep the PE busy while the
            # first DMA chunks land, so the HAM clock-gate is already at
            # 8/8 when the real matmuls start (cold matmuls run at 1.2GHz).
            warm_sb = consts.tile([128, 64], DT, name="warm_sb")
            nc.vector.memset(warm_sb, 1.0)
            warm_ps = nar("warm_ps")
            for _ in range(20):
                nc.tensor.matmul(
                    warm_ps[0:64, 0:64], lhsT=warm_sb[:, 0:64],
                    rhs=warm_sb[:, 0:64], start=True, stop=True,
                    skip_group_check=True,
                )

            states = {}
            states[0] = emit_proj(0)
            emit_evac(0, states[0])
            emit_rope(0, states[0])
            for tb in range(tb_n):
                if tb + 1 < tb_n:
                    states[tb + 1] = emit_proj(tb + 1)
                    emit_evac(tb + 1, states[tb + 1])
                emit_attn(tb, states[tb])
                if tb + 1 < tb_n:
                    emit_rope(tb + 1, states[tb + 1])
                    states.pop(tb)

            while oproj_units:
                emit_oproj_unit("s", split_dma=True)
                emit_oproj_unit("v", split_dma=True)

    nc.compile()
    return nc


def shard_inputs(x, cos, sin, Wq, bq, Wkv, bkv, Wo, t=T, dt16="bf16"):
    """Build the 8 per-core input maps (core c -> batch c//4, group c%4)."""
    import ml_dtypes

    DT = {"bf16": ml_dtypes.bfloat16, "fp16": np.float16}[dt16]
    f32 = np.float32
    hs = HS
    rot = np.zeros((hs, hs), f32)
    for i in range(hs // 2):
        rot[i, i + hs // 2] = -1.0
        rot[i + hs // 2, i] = 1.0
    r_t = np.ascontiguousarray(rot.T).astype(DT)
    mask_ut = np.triu(np.ones((128, 128), f32)).astype(DT)
    ident = np.eye(128, dtype=f32)
    cos_t = np.ascontiguousarray(cos.T).astype(DT)
    sin_t = np.ascontiguousarray(sin.T).astype(DT)

    xts = [np.ascontiguousarray(x[b].T).astype(DT) for b in range(x.shape[0])]
    per_g = []
    for g in range(4):
        per_g.append(
            dict(
                wq_t=np.ascontiguousarray(
                    Wq[512 * g: 512 * g + 512].T).astype(DT),
                b_q=np.ascontiguousarray(
                    bq[512 * g: 512 * g + 512].reshape(4, 128).T.astype(f32)
                ),
                wk_t=np.ascontiguousarray(
                    Wkv[128 * g: 128 * g + 128].T).astype(DT),
                b_k=np.ascontiguousarray(
                    bkv[128 * g: 128 * g + 128].reshape(128, 1).astype(f32)
                ),
                wv_t=np.ascontiguousarray(
                    Wkv[512 + 128 * g: 512 + 128 * g + 128].T).astype(DT),
                b_v=np.ascontiguousarray(
                    bkv[512 + 128 * g: 512 + 128 * g + 128]
                    .reshape(128, 1)
                    .astype(f32)
                ),
                wo_t=np.ascontiguousarray(
                    Wo[:, 512 * g: 512 * g + 512].T).astype(DT),
            )
        )

    in_maps = []
    for c in range(4 * x.shape[0]):
        b, g = c // 4, c % 4
        m = dict(per_g[g])
        m.update(
            x_t=xts[b], cos_t=cos_t, sin_t=sin_t,
            r_t=r_t, mask_ut=mask_ut, ident=ident,
        )
        in_maps.append(m)
    return in_maps


def run_on_hw(in_maps, t=T, trace=False, **flags):
    from concourse.bass_utils import run_bass_kernel_spmd

    key = (t, tuple(sorted(flags.items())))
    if key not in _NC_CACHE:
        _NC_CACHE[key] = build_nc(t, **flags)
    nc = _NC_CACHE[key]
    res = run_bass_kernel_spmd(
        nc, in_maps, core_ids=list(range(len(in_maps))), trace=trace
    )
    return res


def kernel(x, cos, sin, Wq, bq, Wkv, bkv, Wo):
    x = np.asarray(x)
    in_maps = shard_inputs(
        x, np.asarray(cos), np.asarray(sin), np.asarray(Wq), np.asarray(bq),
        np.asarray(Wkv), np.asarray(bkv), np.asarray(Wo),
    )
    res = run_on_hw(in_maps, t=T, trace=False)
    out = np.zeros((B, T, D), np.float32)
    for c, rmap in enumerate(res.results):
        out[c // 4] += rmap["out"]
    return out


# revision 57
# speedup vs baseline: 1.0118x; 1.0026x over previous
"""Causal group-query attention on 8 Trainium2 NeuronCores (bf16 edition).

Sharding: core c -> (batch b = c // 4, kv-group g = c % 4).
Each core owns batch element b, q-heads [4g, 4g+4) and kv-group g (n_rep = 4,
so those 4 q-heads attend to exactly kv-group g's k/v).  Every core computes
its partial o_proj output (contracting head-concat columns [512g, 512g+512)),
and the host sums the 4 partials per batch element (the "all-reduce after
o_proj" done host-side since we return full outputs anyway).

v3 (~317us) vs the fp32r baseline (394us) -- what moved the needle:
  * all matmuls in bf16.  NOTE: measured on HW, bf16 does NOT stream
    2 elem/cycle on the PE (fp32r was already at the 1 col/cycle issue
    rate); bf16 still wins ~7% PE time via FWL weight loads + halved
    SBUF/HBM traffic, and halves the DVE elementwise cost.
  * LDWEIGHTS sharing: consecutive matmuls that reuse the same
    stationary operand are emitted adjacently (both dens share ones16,
    both PVs share the v tile, o_proj runs h-outer over db-pairs so each
    osb slice loads once per pair).  This alone was worth ~17us.
  * software-pipelined schedule: proj(tb+1) is emitted before attn(tb),
    and rope(tb+1) after it, so the PE's projection stream covers the
    serial bias-evac/RoPE chains on ACT/DVE, and deferred o_proj units
    of block tb-1 fill the rot-matmul and pair-tail waits.
  * PSUM: 2 wide [128,2,512] tiles + 4 narrow [128,512] tiles (8 banks).
    Phase A: qt01/qt23 wide, kt/vt narrow.  Phase B: the wide tiles
    double-buffer the head-pair score banks, so one ACT instruction
    computes exp for BOTH heads of a pair ([128,2,N] 3-D AP), halving
    ACT's 352-cycle per-instruction overhead.
  * softmax denominators: ones-matmul per (head, k-tile) into a [1,512]
    PSUM row; reciprocal via reciprocal_approx_fast straight from PSUM
    (~5x faster than nc.vector.reciprocal), PE ones-broadcast of the
    reciprocal, DVE multiply reading the broadcast directly from PSUM.
  * q bias-evacs on ACT, k/v on DVE (the two chains overlap); causal
    masking on the otherwise-idle GpSimd; one-time bulk DMAs gated
    behind x-chunk 3 so they don't starve the critical first chunks;
    wide [128,2048] output tiles shipped with one DMA each, alternating
    gpsimd/sync queues (split per-half for the tail units); PE warm-up
    matmuls while the first DMAs land (HAM clock-gate).
Known-bad variants (do not revisit): den rows packed into one bank via
tile_position=(0,32) -> silently wrong on HW; kt/vt sharing a wide tile
with a 3-wide/2-narrow PSUM split -> slower (ring too tight).
"""

import math

import numpy as np

B, T, D = 2, 2048, 2048
N_HEAD, N_GROUP = 16, 4
HS = D // N_HEAD  # 128
N_REP = N_HEAD // N_GROUP  # 4
NH_C = N_HEAD // N_GROUP  # heads per core = 4
INV_SQRT_HS = 1.0 / math.sqrt(HS)

_NC_CACHE: dict = {}


def build_nc(t=T, dt16="bf16", recip_psum=True, gp_mask=True,
             pair_oproj=True):
    """Build and compile the per-core Bass program. Returns the compiled nc."""
    import concourse.bass as bass  # noqa: F401
    import concourse.mybir as mybir
    import concourse.tile as tile
    from concourse import bacc

    f32 = mybir.dt.float32
    f32r = mybir.dt.float32r
    DT = {"bf16": mybir.dt.bfloat16, "fp16": mybir.dt.float16}[dt16]
    exp_f = mybir.ActivationFunctionType.Exp
    ident_f = mybir.ActivationFunctionType.Identity



    nd = D // 128  # d-tiles (contraction) = 16
    tb_n = t // 512  # 512-wide t blocks
    nk = t // 128  # 128-wide k tiles

    nc = bacc.Bacc("TRN2", target_bir_lowering=False, debug=False)

    xd = nc.dram_tensor("x_t", [D, t], DT, kind="ExternalInput")
    wqd = nc.dram_tensor("wq_t", [D, NH_C * HS], DT, kind="ExternalInput")
    wkd = nc.dram_tensor("wk_t", [D, HS], DT, kind="ExternalInput")
    wvd = nc.dram_tensor("wv_t", [D, HS], DT, kind="ExternalInput")
    wod = nc.dram_tensor("wo_t", [NH_C * HS, D], DT, kind="ExternalInput")
    cosd = nc.dram_tensor("cos_t", [HS, t], DT, kind="ExternalInput")
    sind = nc.dram_tensor("sin_t", [HS, t], DT, kind="ExternalInput")
    bqd = nc.dram_tensor("b_q", [HS, NH_C], f32, kind="ExternalInput")
    bkd = nc.dram_tensor("b_k", [HS, 1], f32, kind="ExternalInput")
    bvd = nc.dram_tensor("b_v", [HS, 1], f32, kind="ExternalInput")
    rtd = nc.dram_tensor("r_t", [HS, HS], DT, kind="ExternalInput")
    maskd = nc.dram_tensor("mask_ut", [128, 128], DT, kind="ExternalInput")
    identd = nc.dram_tensor("ident", [128, 128], f32, kind="ExternalInput")
    outd = nc.dram_tensor("out", [t, D], f32, kind="ExternalOutput")

    with tile.TileContext(nc) as tc:
        with (
            tc.tile_pool(name="consts", bufs=1) as consts,
            tc.tile_pool(name="wpool", bufs=1) as wpool,
            tc.tile_pool(name="resid", bufs=1) as resid,
            tc.tile_pool(name="xin", bufs=10) as xin,
            tc.tile_pool(name="work", bufs=4) as work,
            tc.tile_pool(name="vwork", bufs=2) as vwork,
            tc.tile_pool(name="drow", bufs=4) as drow,
            tc.tile_pool(name="qfp", bufs=2) as qfp,
            tc.tile_pool(name="ptp", bufs=6) as ptp,
            tc.tile_pool(name="otp", bufs=8) as otp,
            tc.tile_pool(name="bcp", bufs=2) as bcp,
            tc.tile_pool(name="outp", bufs=4) as outp,
            tc.tile_pool(name="psw", bufs=2, space="PSUM") as psw,
            tc.tile_pool(name="psn", bufs=4, space="PSUM") as psn,
        ):
            def wide(name):
                return psw.tile([128, 2, 512], f32, tag="wide", name=name)

            def nar(name):
                return psn.tile([128, 512], f32, tag="nar", name=name)

            # ---- constants / weights (loaded once) ----
            cos_sb = consts.tile([128, t], DT, name="cos_sb")
            sin_sb = consts.tile([128, t], DT, name="sin_sb")
            rt_sb = consts.tile([128, 128], DT, name="rt_sb")
            mask_sb = consts.tile([128, 128], DT, name="mask_sb")
            id_sb = consts.tile([128, 128], f32, name="id_sb")
            ones16 = consts.tile([128, 1], DT, name="ones16")
            onesf_raw = consts.tile([1, 128], f32, name="onesf_raw")
            onesf = consts.tile([1, 128], f32, name="onesf")
            bq_sb = consts.tile([128, NH_C], f32, name="bq_sb")
            bk_sb = consts.tile([128, 1], f32, name="bk_sb")
            bv_sb = consts.tile([128, 1], f32, name="bv_sb")
            wq_sb = wpool.tile([128, nd, NH_C * HS], DT, name="wq_sb")
            wk_sb = wpool.tile([128, nd, HS], DT, name="wk_sb")
            wv_sb = wpool.tile([128, nd, HS], DT, name="wv_sb")
            wo_sb = wpool.tile([128, NH_C, D], DT, name="wo_sb")
            wq_re = wqd[:, :].rearrange("(n p) m -> p n m", p=128)
            wk_re = wkd[:, :].rearrange("(n p) m -> p n m", p=128)
            wv_re = wvd[:, :].rearrange("(n p) m -> p n m", p=128)

            # resident K^T [hs, t] and V [t(128-tiles), hs]
            kt_sb = resid.tile([128, t], DT, name="kt_sb")
            v_sb = resid.tile([128, nk, HS], DT, name="v_sb")

            x_re = xd[:, :].rearrange("(n p) t -> p n t", p=128)

            oproj_units = []  # deferred (tb, s, ot_sb) work units
            oproj_q = [0]  # alternate output-DMA queue (gpsimd / sync)

            def emit_oproj_unit(evac_eng, split_dma=False):
                # one unit = a full 128-row output block: 4 db sub-blocks of
                # 4 head-accumulated matmuls each, evacuated into one wide
                # [128, 2048] tile and shipped with a single DMA.  With
                # split_dma (tail units) each db half ships separately so the
                # transfer overlaps the remaining evacuations.
                if not oproj_units:
                    return
                tb, s, ot_sb = oproj_units.pop(0)
                ts0 = tb * 512
                ob = outp.tile([128, D], f32, name="ob")
                # db-pairs with h outer: each head's stationary osb slice is
                # loaded once per pair of db sub-blocks instead of per block.
                for dbp in range(2):
                    op_ps = {db: nar("op_ps") for db in (2 * dbp, 2 * dbp + 1)}
                    for h in range(NH_C):
                        for db in (2 * dbp, 2 * dbp + 1):
                            nc.tensor.matmul(
                                op_ps[db],
                                lhsT=ot_sb[h][:, 128 * s: 128 * (s + 1)],
                                rhs=wo_sb[:, h, 512 * db: 512 * (db + 1)],
                                start=h == 0, stop=h == NH_C - 1,
                            )
                    for db in (2 * dbp, 2 * dbp + 1):
                        ob_sl = ob[:, 512 * db: 512 * (db + 1)]
                        if (db % 2 == 0) == (evac_eng == "v"):
                            nc.vector.tensor_copy(out=ob_sl, in_=op_ps[db])
                        else:
                            nc.scalar.copy(out=ob_sl, in_=op_ps[db])
                    if split_dma:
                        eng = nc.gpsimd if oproj_q[0] % 2 == 0 else nc.sync
                        oproj_q[0] += 1
                        eng.dma_start(
                            out=outd[
                                ts0 + 128 * s: ts0 + 128 * (s + 1),
                                1024 * dbp: 1024 * (dbp + 1),
                            ],
                            in_=ob[:, 1024 * dbp: 1024 * (dbp + 1)],
                        )
                if not split_dma:
                    eng = nc.gpsimd if oproj_q[0] % 2 == 0 else nc.sync
                    oproj_q[0] += 1
                    eng.dma_start(
                        out=outd[ts0 + 128 * s: ts0 + 128 * (s + 1), :],
                        in_=ob,
                    )

            def emit_proj(tb):
                # ============ stage 1: q/k/v projection matmuls for block tb
                ts0 = tb * 512
                qt01 = wide("qt01")
                qt23 = wide("qt23")
                kt_ps = nar("kt_ps")
                vt_ps = nar("vt_ps")
                xts_tb = []
                for chunk in range(nd // 2):
                    c2 = 2 * chunk
                    xt = xin.tile([128, 2, 512], DT, name="xt")
                    xts_tb.append(xt)
                    nc.sync.dma_start(
                        out=xt, in_=x_re[:, c2: c2 + 2, ts0: ts0 + 512]
                    )
                    if tb == 0:
                        nc.sync.dma_start(
                            out=wq_sb[:, c2: c2 + 2, :],
                            in_=wq_re[:, c2: c2 + 2, :],
                        )
                        nc.sync.dma_start(
                            out=wk_sb[:, c2: c2 + 2, :],
                            in_=wk_re[:, c2: c2 + 2, :],
                        )
                        nc.sync.dma_start(
                            out=wv_sb[:, c2: c2 + 2, :],
                            in_=wv_re[:, c2: c2 + 2, :],
                        )
                        if chunk == 0:
                            nc.sync.dma_start(out=bq_sb, in_=bqd[:, :])
                            nc.sync.dma_start(out=bk_sb, in_=bkd[:, :])
                            nc.sync.dma_start(out=bv_sb, in_=bvd[:, :])
                            nc.vector.memset(ones16, 1.0)
                            nc.vector.memset(onesf_raw, 1.0)
                            # fp32r matmul operands must come from a
                            # producer with f32r-typed output (walrus rule)
                            nc.vector.tensor_copy(
                                out=onesf[:, :].bitcast(f32r), in_=onesf_raw
                            )
                    for j in range(2):
                        dt = c2 + j
                        first, last = dt == 0, dt == nd - 1
                        for h in range(2):
                            nc.tensor.matmul(
                                qt01[:, h, :],
                                lhsT=wq_sb[:, dt, h * HS: (h + 1) * HS],
                                rhs=xt[:, j, :],
                                start=first, stop=last,
                            )
                        for h in range(2):
                            nc.tensor.matmul(
                                qt23[:, h, :],
                                lhsT=wq_sb[:, dt, (2 + h) * HS: (3 + h) * HS],
                                rhs=xt[:, j, :],
                                start=first, stop=last,
                            )
                        nc.tensor.matmul(
                            kt_ps, lhsT=wk_sb[:, dt, :], rhs=xt[:, j, :],
                            start=first, stop=last,
                        )
                        nc.tensor.matmul(
                            vt_ps, lhsT=wv_sb[:, dt, :], rhs=xt[:, j, :],
                            start=first, stop=last,
                        )

                if tb == 0:
                    # one-time loads go on the gpsimd queue so the sync queue
                    # stays dedicated to the xt stream.  Gate the queue on
                    # chunk 3's arrival so these bulk loads don't steal HBM
                    # bandwidth from the critical first x/w chunks.
                    gate = drow.tile([1, 1], DT, name="gate")
                    nc.gpsimd.tensor_copy(
                        out=gate, in_=xts_tb[3][0:1, 0, 0:1]
                    )
                    nc.gpsimd.dma_start(out=rt_sb, in_=rtd[:, :])
                    nc.gpsimd.dma_start(out=id_sb, in_=identd[:, :])
                    nc.gpsimd.dma_start(out=mask_sb, in_=maskd[:, :])
                    nc.gpsimd.dma_start(out=cos_sb, in_=cosd[:, :])
                    nc.gpsimd.dma_start(out=sin_sb, in_=sind[:, :])
                    wo_re = wod[:, :].rearrange("(h p) m -> p h m", p=128)
                    for h in range(NH_C):
                        nc.gpsimd.dma_start(
                            out=wo_sb[:, h: h + 1, :],
                            in_=wo_re[:, h: h + 1, :],
                        )
                return dict(qt01=qt01, qt23=qt23, kt_ps=kt_ps, vt_ps=vt_ps)

            def emit_evac(tb, stt):
                # ============ stage 2: bias-add PSUM evacuations.  k/v on
                # DVE (the k-rope chain is the critical path), q on ACT so
                # the two chains overlap.  Frees all of stage 1's banks.
                kraw = work.tile([128, 512], DT, name="kraw", bufs=2)
                nc.vector.tensor_scalar_add(
                    out=kraw, in0=stt["kt_ps"], scalar1=bk_sb[:, 0:1]
                )
                vraw = vwork.tile([128, 512], f32, name="vraw")
                nc.vector.tensor_scalar_add(
                    out=vraw, in0=stt["vt_ps"], scalar1=bv_sb[:, 0:1]
                )
                qraws = []
                for h in range(NH_C):
                    qraw = work.tile([128, 512], DT, name="qraw")
                    src = (stt["qt01"][:, h, :] if h < 2
                           else stt["qt23"][:, h - 2, :])
                    nc.scalar.activation(
                        out=qraw, in_=src, func=ident_f,
                        bias=bq_sb[:, h: h + 1], scale=1.0,
                    )
                    qraws.append(qraw)
                stt.update(kraw=kraw, vraw=vraw, qraws=qraws)

            def emit_rope(tb, stt):
                # ============ stage 3: RoPE (k then q) + V transpose.
                # o_proj units of the previous block keep the PE fed while
                # ACT/DVE chew the serial chains.
                ts0 = tb * 512
                kraw, vraw, qraws = stt["kraw"], stt["vraw"], stt["qraws"]
                qf = qfp.tile([128, NH_C, 512], DT, name="qf")
                rot_k = nar("rot_ps")
                nc.tensor.matmul(
                    rot_k, lhsT=rt_sb[:, :], rhs=kraw, start=True, stop=True
                )
                if not pair_oproj:
                    emit_oproj_unit("s")
                nc.vector.tensor_mul(
                    kt_sb[:, ts0: ts0 + 512], kraw, cos_sb[:, ts0: ts0 + 512]
                )
                rtmp = work.tile([128, 512], DT, name="rtmp", bufs=2)
                nc.vector.tensor_mul(rtmp, rot_k, sin_sb[:, ts0: ts0 + 512])
                nc.vector.tensor_add(
                    kt_sb[:, ts0: ts0 + 512], kt_sb[:, ts0: ts0 + 512], rtmp
                )

                # v: transpose [hs, t] -> [t, hs] tiles
                for s in range(4):
                    vt_tp = nar("vt_tp")
                    nc.tensor.transpose(
                        vt_tp[:, 0:128], vraw[:, 128 * s: 128 * (s + 1)],
                        id_sb[:, :],
                    )
                    nc.vector.tensor_copy(
                        out=v_sb[:, 4 * tb + s, :], in_=vt_tp[:, 0:128]
                    )

                rot_ps = {}
                for h in range(NH_C):
                    rot_ps[h] = nar("rot_ps")
                    nc.tensor.matmul(
                        rot_ps[h], lhsT=rt_sb[:, :], rhs=qraws[h],
                        start=True, stop=True,
                    )
                    if h % 2 == 1:
                        emit_oproj_unit("v" if h == 1 else "s")
                    nc.vector.tensor_mul(
                        qf[:, h, :], qraws[h], cos_sb[:, ts0: ts0 + 512]
                    )
                    rtmp = work.tile([128, 512], DT, name="rtmp", bufs=2)
                    nc.vector.tensor_mul(
                        rtmp, rot_ps[h], sin_sb[:, ts0: ts0 + 512]
                    )
                    nc.vector.tensor_add(qf[:, h, :], qf[:, h, :], rtmp)
                if not pair_oproj:
                    emit_oproj_unit("v")
                stt["qf"] = qf

            def emit_attn(tb, stt):
                # ============ stage 4: attention for q-block jq == tb
                # Heads in pairs; both heads' scores live in one wide PSUM
                # tile so exp is a single [128, 2, N] ACT op.  PV/denominator
                # of k-tile i-1 issue while ACT computes exp of tile i.
                qf = stt["qf"]
                ot_sb = {}
                imax = 4 * tb + 3
                for hp in range(NH_C // 2):
                    heads = (2 * hp, 2 * hp + 1)
                    ot_ps = {h: nar(f"ot_ps{h}") for h in heads}
                    # NOTE: packing both heads' den rows into one bank (via
                    # tile_position=(0,32) for head 1) was tried and gives
                    # silently WRONG results on hardware -- keep one bank and
                    # partition row 0 per head.
                    dps = {h: nar(f"den_ps{h}") for h in heads}
                    den_ap = {h: dps[h][0:1, :] for h in heads}

                    def emit_pv_den(i, pt, c0):
                        # dens first (reciprocal chain starts earlier) and
                        # adjacent (they share the ones16 stationary operand),
                        # then both pvs (they share the v tile).
                        first, last = i == 0, i == imax
                        for j, h in enumerate(heads):
                            nc.tensor.matmul(
                                den_ap[h][:, c0:],
                                lhsT=ones16[:, 0:1],
                                rhs=pt[:, j, c0:],
                                start=first, stop=last,
                                skip_group_check=True,
                            )
                        for j, h in enumerate(heads):
                            nc.tensor.matmul(
                                ot_ps[h][:, c0:],
                                lhsT=v_sb[:, i, :],
                                rhs=pt[:, j, c0:],
                                start=first, stop=last,
                            )

                    prev = None
                    for i in range(imax + 1):
                        c0 = 128 * max(0, i - 4 * tb)
                        diag = i >= 4 * tb
                        st = wide("st")
                        for j, h in enumerate(heads):
                            nc.tensor.matmul(
                                st[:, j, c0:],
                                lhsT=kt_sb[:, 128 * i: 128 * (i + 1)],
                                rhs=qf[:, h, c0:],
                                start=True, stop=True,
                            )
                        pt = ptp.tile([128, 2, 512], DT, name="pt")
                        nc.scalar.activation(
                            out=pt[:, :, c0:], in_=st[:, :, c0:], func=exp_f,
                            scale=INV_SQRT_HS,
                        )
                        if diag:
                            for j in range(2):
                                if gp_mask:
                                    # zero strictly-lower triangle (tk > tq)
                                    nc.gpsimd.affine_select(
                                        out=pt[:, j, c0: c0 + 128],
                                        in_=pt[:, j, c0: c0 + 128],
                                        compare_op=mybir.AluOpType.is_ge,
                                        fill=0.0,
                                        base=0,
                                        pattern=[[1, 128]],
                                        channel_multiplier=-1,
                                    )
                                else:
                                    nc.vector.tensor_mul(
                                        pt[:, j, c0: c0 + 128],
                                        pt[:, j, c0: c0 + 128],
                                        mask_sb,
                                    )
                        if prev is not None:
                            emit_pv_den(*prev)
                        prev = (i, pt, c0)
                    emit_pv_den(*prev)

                    # normalize each head's O^T by its softmax denominator:
                    # 1/den straight from PSUM (reciprocal_approx_fast) frees
                    # the den banks at once; ACT evacuates O^T (freeing the ot
                    # banks for the next pair); PE broadcasts the reciprocal
                    # across partitions and DVE multiplies it in-place,
                    # reading the broadcast directly from PSUM.
                    denrec = {}
                    for j, h in enumerate(heads):
                        draw = drow.tile([1, 512], f32, name="draw")
                        if recip_psum:
                            nc.vector.reciprocal_approx_fast(
                                out=draw, in_=den_ap[h]
                            )
                        else:
                            dcp = drow.tile([1, 512], f32, name="dcp")
                            nc.scalar.copy(out=dcp, in_=den_ap[h])
                            nc.vector.reciprocal_approx_fast(
                                out=draw, in_=dcp
                            )
                        denrec[h] = drow.tile([1, 512], f32, name="denrec")
                        nc.vector.tensor_copy(
                            out=denrec[h][:, :].bitcast(f32r), in_=draw
                        )
                    for j, h in enumerate(heads):
                        osb = otp.tile([128, 512], DT, name="osb")
                        nc.scalar.copy(out=osb, in_=ot_ps[h])
                        ot_sb[h] = osb
                    for j, h in enumerate(heads):
                        bc_ps = nar("bc_ps")
                        nc.tensor.matmul(
                            bc_ps,
                            lhsT=onesf[:, :].bitcast(f32r),
                            rhs=denrec[h][:, :].bitcast(f32r),
                            start=True, stop=True,
                        )
                        nc.vector.tensor_mul(ot_sb[h], ot_sb[h], bc_ps)
                    if pair_oproj:
                        # all pair banks are released here: an o_proj unit
                        # fills the PE while the next pair's chains warm up
                        emit_oproj_unit("s" if hp == 0 else "v")

                for s in range(4):
                    oproj_units.append((tb, s, ot_sb))

            # ============ pipelined schedule: projections of block tb+1 are
            # emitted before attention of block tb, so the PE's projection
            # stream covers the serial evac/rope chains on ACT/DVE, and the
            # rope of tb+1 lands after attention of tb where the previous
            # block's o_proj units fill the rot-matmul waits.
            # warm-up: dependency-free matmuls keep the PE busy while the
            # first DMA chunks land, so the HAM clock-gate is already at
            # 8/8 when the real matmuls start (cold matmuls run at 1.2GHz).
            warm_sb = consts.tile([128, 64], DT, name="warm_sb")
            nc.vector.memset(warm_sb, 1.0)
            warm_ps = nar("warm_ps")
            for _ in range(20):
                nc.tensor.matmul(
                    warm_ps[0:64, 0:64], lhsT=warm_sb[:, 0:64],
                    rhs=warm_sb[:, 0:64], start=True, stop=True,
                    skip_group_check=True,
                )

            states = {}
            states[0] = emit_proj(0)
            emit_evac(0, states[0])
            emit_rope(0, states[0])
            for tb in range(tb_n):
                if tb + 1 < tb_n:
                    states[tb + 1] = emit_proj(tb + 1)
                    emit_evac(tb + 1, states[tb + 1])
                emit_attn(tb, states[tb])
                if tb + 1 < tb_n:
                    emit_rope(tb + 1, states[tb + 1])
                    states.pop(tb)

            while oproj_units:
                emit_oproj_unit("s", split_dma=True)
                emit_oproj_unit("v", split_dma=True)

    nc.compile()
    return nc


def shard_inputs(x, cos, sin, Wq, bq, Wkv, bkv, Wo, t=T, dt16="bf16"):
    """Build the 8 per-core input maps (core c -> batch c//4, group c%4)."""
    import ml_dtypes

    DT = {"bf16": ml_dtypes.bfloat16, "fp16": np.float16}[dt16]
    f32 = np.float32
    hs = HS
    rot = np.zeros((hs, hs), f32)
    for i in range(hs // 2):
        rot[i, i + hs // 2] = -1.0
        rot[i + hs // 2, i] = 1.0
    r_t = np.ascontiguousarray(rot.T).astype(DT)
    mask_ut = np.triu(np.ones((128, 128), f32)).astype(DT)
    ident = np.eye(128, dtype=f32)
    cos_t = np.ascontiguousarray(cos.T).astype(DT)
    sin_t = np.ascontiguousarray(sin.T).astype(DT)

    xts = [np.ascontiguousarray(x[b].T).astype(DT) for b in range(x.shape[0])]
    per_g = []
    for g in range(4):
        per_g.append(
            dict(
                wq_t=np.ascontiguousarray(
                    Wq[512 * g: 512 * g + 512].T).astype(DT),
                b_q=np.ascontiguousarray(
                    bq[512 * g: 512 * g + 512].reshape(4, 128).T.astype(f32)
                ),
                wk_t=np.ascontiguousarray(
                    Wkv[128 * g: 128 * g + 128].T).astype(DT),
                b_k=np.ascontiguousarray(
                    bkv[128 * g: 128 * g + 128].reshape(128, 1).astype(f32)
                ),
                wv_t=np.ascontiguousarray(
                    Wkv[512 + 128 * g: 512 + 128 * g + 128].T).astype(DT),
                b_v=np.ascontiguousarray(
                    bkv[512 + 128 * g: 512 + 128 * g + 128]
                    .reshape(128, 1)
                    .astype(f32)
                ),
                wo_t=np.ascontiguousarray(
                    Wo[:, 512 * g: 512 * g + 512].T).astype(DT),
            )
        )

    in_maps = []
    for c in range(4 * x.shape[0]):
        b, g = c // 4, c % 4
        m = dict(per_g[g])
        m.update(
            x_t=xts[b], cos_t=cos_t, sin_t=sin_t,
            r_t=r_t, mask_ut=mask_ut, ident=ident,
        )
        in_maps.append(m)
    return in_maps


def run_on_hw(in_maps, t=T, trace=False, **flags):
    from concourse.bass_utils import run_bass_kernel_spmd

    key = (t, tuple(sorted(flags.items())))
    if key not in _NC_CACHE:
        _NC_CACHE[key] = build_nc(t, **flags)
    nc = _NC_CACHE[key]
    res = run_bass_kernel_spmd(
        nc, in_maps, core_ids=list(range(len(in_maps))), trace=trace
    )
    return res


def kernel(x, cos, sin, Wq, bq, Wkv, bkv, Wo):
    x = np.asarray(x)
    in_maps = shard_inputs(
        x, np.asarray(cos), np.asarray(sin), np.asarray(Wq), np.asarray(bq),
        np.asarray(Wkv), np.asarray(bkv), np.asarray(Wo),
    )
    res = run_on_hw(in_maps, t=T, trace=False)
    out = np.zeros((B, T, D), np.float32)
    for c, rmap in enumerate(res.results):
        out[c // 4] += rmap["out"]
    return out
